# revision 24
# baseline (speedup 1.0000x reference)
"""GroupFC kernel for Trainium2, data-parallel across 8 NeuronCores.

Problem: out = data @ W.T + b
  data: [32768, 1024] f32, W: [1024, 1024] f32, b: [1024] f32

Strategy:
  - Shard batch dim across 8 cores (4096 rows each); replicate W, b.
  - Transposed-output formulation: outT[o, b] = sum_k W[o,k] d[b,k] + b[o].
    Stationary operand = W tiles (out-dim on PSUM partitions), moving
    operand = data columns (batch on the free dim).
  - Mixed precision along the contraction, tuned to the 2e-2 rel-err
    budget: two batch quarters (q1, q2) run k-blocks 0..5 in bf16
    (1 col/cycle) and blocks 6,7 in fp8-e4m3 DoubleRow (measured ~2x
    column rate); the other two (q0, q3) run blocks 0..3 bf16 and 4..7
    fp8 (two DR pairs). Measured rel err ~1.87e-2.
  - q0 is a deep-fp8 quarter on purpose: the DR-first ramp phase gets
    ~2x the fp8 work from ~1 MiB of loads, so the PE is busy while the
    bf16 tiles stream in, and the bf16 k-outer ramp is 4 steps, not 6.
  - All W values pre-scaled by 128 on the host so the fp8 weights avoid
    the e4m3 subnormal range; the fused evacuation applies 1/128 and the
    per-out-row bias in one pass per bank (DVE; ACT joins for the last
    quarter), emitting bf16 halves stored immediately on both HWDGE
    rings.
  - Startup: memset-gated warmup matmuls ramp the PE HAM clock gate.
  - Host post-pass transposes outT back to [batch, out] f32.
"""

import sys
from contextlib import ExitStack

import numpy as np

try:
    import concourse.bass as bass  # noqa: F401
except ImportError:
    sys.path.insert(0, "/opt/trn_rl_repo")

import ml_dtypes

import concourse.tile as tile
from concourse import bacc, mybir
from concourse.bass_utils import run_bass_kernel_spmd

N_CORES = 8
BATCH = 32768
SHARD = BATCH // N_CORES  # 4096
IN_DIM = 1024
OUT_DIM = 1024
P = 128
KB = 6  # bf16 k-blocks for shallow quarters; deep quarters use KB-2
NQ = 4  # batch quarters per core (1024 columns each)
QCOL = SHARD // NQ  # 1024
NO = OUT_DIM // P  # 8 output-row blocks
SCALE = 128.0
DEEP = (True, False, False, True)  # per-quarter: 4 fp8 k-blocks vs 2
E4 = ml_dtypes.float8_e4m3
BF = ml_dtypes.bfloat16

_CACHE = {}


def _build():
    nc = bacc.Bacc("TRN2", target_bir_lowering=False, debug=False)
    dT = nc.dram_tensor(
        "dT", [KB, P, SHARD], mybir.dt.bfloat16, kind="ExternalInput"
    ).ap()
    d8a = nc.dram_tensor(  # shallow quarters (q1, q2), k-blocks 6,7
        "d8a", [2, P, 2, QCOL], mybir.dt.float8e4, kind="ExternalInput"
    ).ap()
    d8b = nc.dram_tensor(  # deep quarters (q0, q3), k-blocks 4..7
        "d8b", [2, P, 4, QCOL], mybir.dt.float8e4, kind="ExternalInput"
    ).ap()
    wT = nc.dram_tensor(
        "wT", [KB, P, OUT_DIM], mybir.dt.bfloat16, kind="ExternalInput"
    ).ap()
    w8hi = nc.dram_tensor(  # fp8 weights, k-blocks 6,7
        "w8hi", [P, 2, OUT_DIM], mybir.dt.float8e4, kind="ExternalInput"
    ).ap()
    w8lo = nc.dram_tensor(  # fp8 weights, k-blocks 4,5
        "w8lo", [P, 2, OUT_DIM], mybir.dt.float8e4, kind="ExternalInput"
    ).ap()
    biasb = nc.dram_tensor(
        "biasb", [P, NO], mybir.dt.float32, kind="ExternalInput"
    ).ap()
    outT = nc.dram_tensor(
        "outT", [OUT_DIM, SHARD], mybir.dt.bfloat16, kind="ExternalOutput"
    ).ap()

    with tile.TileContext(nc) as tc:
        with ExitStack() as ctx:
            wp = ctx.enter_context(tc.tile_pool(name="w", bufs=1))
            dp = ctx.enter_context(tc.tile_pool(name="d", bufs=1))
            bp = ctx.enter_context(tc.tile_pool(name="misc", bufs=1))
            pp = ctx.enter_context(tc.tile_pool(name="psum", bufs=4, space="PSUM"))
            op = ctx.enter_context(tc.tile_pool(name="o", bufs=8))

            w_t = [None] * KB
            d_t = [[None] * NQ for _ in range(KB)]
            d8_t = [None] * NQ  # q0 uses the half tiles below instead
            tiles = {}

            def kmax(q):
                return KB - 2 if DEEP[q] else KB

            # Load plan. Critical ramp first: the fp8 weights (both pairs)
            # and the q0 fp8 data halves unlock the DR-first phase from
            # ~1.1 MiB; then (wT[k], dT[k] q0) pairs k=0..3 in consumption
            # order; then the rest. Alternate the two HWDGE rings.
            loads = [
                ("bias", 0, 0), ("w8hiA", 0, 0), ("d8q0A", 0, 0),
                ("w8lo", 0, 0), ("d8q0B", 0, 0), ("w8hiB", 0, 0),
            ]
            for k in range(4):
                loads.append(("w", k, 0))
                loads.append(("d", k, 0))
            loads += [("w", 4, 0), ("w", 5, 0)]
            for q in range(1, NQ):
                for k in range(kmax(q)):
                    loads.append(("d", k, q))
                loads.append(("d8", 0, q))

            for i, (kind, k, q) in enumerate(loads):
                eng = nc.scalar if i % 2 == 0 else nc.sync
                if kind == "w":
                    w_t[k] = wp.tile([P, OUT_DIM], mybir.dt.bfloat16, tag=f"w{k}", name=f"w_t{k}")
                    eng.dma_start(out=w_t[k][:], in_=wT[k, :, :])
                elif kind == "d":
                    d_t[k][q] = dp.tile([P, QCOL], mybir.dt.bfloat16, tag=f"d{k}_{q}", name=f"d_t{k}_{q}")
                    eng.dma_start(
                        out=d_t[k][q][:], in_=dT[k, :, q * QCOL : (q + 1) * QCOL]
                    )
                elif kind == "w8hiA":
                    tiles["w8hiA"] = wp.tile([P, 2, 512], mybir.dt.float8e4, tag="w8hiA", name="w8hiA_t")
                    eng.dma_start(out=tiles["w8hiA"][:], in_=w8hi[:, :, 0:512])
                elif kind == "w8hiB":
                    tiles["w8hiB"] = wp.tile([P, 2, 512], mybir.dt.float8e4, tag="w8hiB", name="w8hiB_t")
                    eng.dma_start(out=tiles["w8hiB"][:], in_=w8hi[:, :, 512:OUT_DIM])
                elif kind == "w8lo":
                    tiles["w8lo"] = wp.tile([P, 2, OUT_DIM], mybir.dt.float8e4, tag="w8lo", name="w8lo_t")
                    eng.dma_start(out=tiles["w8lo"][:], in_=w8lo[:, :, :])
                elif kind == "d8q0A":
                    tiles["d8q0A"] = dp.tile([P, 4, 512], mybir.dt.float8e4, tag="d8q0A", name="d8q0A_t")
                    eng.dma_start(out=tiles["d8q0A"][:], in_=d8b[0, :, :, 0:512])
                elif kind == "d8q0B":
                    tiles["d8q0B"] = dp.tile([P, 4, 512], mybir.dt.float8e4, tag="d8q0B", name="d8q0B_t")
                    eng.dma_start(out=tiles["d8q0B"][:], in_=d8b[0, :, :, 512:QCOL])
                elif kind == "d8":
                    nblk = 4 if DEEP[q] else 2
                    d8_t[q] = dp.tile([P, nblk, QCOL], mybir.dt.float8e4, tag=f"d8_{q}", name=f"d8_t{q}")
                    src = d8b[1] if DEEP[q] else d8a[q - 1]
                    eng.dma_start(out=d8_t[q][:], in_=src[:, :, :])
                else:
                    bias_t = bp.tile([P, NO], mybir.dt.float32, tag="bias", name="bias_t")
                    eng.dma_start(out=bias_t[:], in_=biasb[:, :])

            # Warmup: ramp the PE HAM clock while loads stream. Gated on an
            # on-chip memset so it starts as soon as the engines come up.
            scr = bp.tile([P, 256], mybir.dt.bfloat16, tag="scr", name="scr")
            nc.vector.memset(scr[:], 0)
            ps_first = [
                pp.tile([P, 512], mybir.dt.float32, tag="pa", name="ps_a0"),
                pp.tile([P, 512], mybir.dt.float32, tag="pb", name="ps_b0"),
            ]
            for i in range(24):
                nc.tensor.matmul(
                    ps_first[0][:, 0:256], scr[:, 0:P], scr[:],
                    start=True, stop=True, skip_group_check=True,
                )

            dr = mybir.MatmulPerfMode.DoubleRow

            def whi(o):
                t = tiles["w8hiA"] if o < 4 else tiles["w8hiB"]
                return t[:, :, (o % 4) * P : (o % 4 + 1) * P]

            def emit_dr(psA, psB, q, o, first):
                # Only the FIRST matmul per bank may set start=True: start
                # clears has_written for the WHOLE bank, so a second start on
                # the other half would wipe the first half's result.
                osl = slice(o * P, (o + 1) * P)
                npair = 2 if DEEP[q] else 1
                for gi in range(npair):
                    if DEEP[q]:
                        wsl = tiles["w8lo"][:, :, osl] if gi == 0 else whi(o)
                        dlo = 2 * gi
                    else:
                        wsl = whi(o)
                        dlo = 0
                    if q == 0:
                        srcs = [
                            tiles["d8q0A"][:, dlo : dlo + 2, 0:256],
                            tiles["d8q0A"][:, dlo : dlo + 2, 256:512],
                            tiles["d8q0B"][:, dlo : dlo + 2, 0:256],
                            tiles["d8q0B"][:, dlo : dlo + 2, 256:512],
                        ]
                    else:
                        dsl = d8_t[q]
                        srcs = [
                            dsl[:, dlo : dlo + 2, 0:256],
                            dsl[:, dlo : dlo + 2, 256:512],
                            dsl[:, dlo : dlo + 2, 512:768],
                            dsl[:, dlo : dlo + 2, 768:QCOL],
                        ]
                    last = gi == npair - 1
                    st = first and gi == 0
                    nc.tensor.matmul(
                        psA[:, 0:256], wsl, srcs[0],
                        start=st, stop=(not first) and last, perf_mode=dr,
                        skip_group_check=True,
                    )
                    nc.tensor.matmul(
                        psA[:, 256:512], wsl, srcs[1],
                        start=False, stop=(not first) and last, perf_mode=dr,
                        skip_group_check=True,
                    )
                    nc.tensor.matmul(
                        psB[:, 0:256], wsl, srcs[2],
                        start=st, stop=(not first) and last, perf_mode=dr,
                        skip_group_check=True,
                    )
                    nc.tensor.matmul(
                        psB[:, 256:512], wsl, srcs[3],
                        start=False, stop=(not first) and last, perf_mode=dr,
                        skip_group_check=True,
                    )

            def emit_evac(psA, psB, q, o):
                # Fused evacuation: out = psum/128 + bias[o], to bf16, with
                # separate half-tiles so each store only waits on its own
                # evacuation and rides its own HW ring. Evacs stay on DVE
                # (the ring queues carry the load DMAs); the last quarter
                # also uses ACT, whose queue has drained by then.
                osl = slice(o * P, (o + 1) * P)
                bcol = bias_t[:, o : o + 1]
                c0 = q * QCOL
                osbA = op.tile([P, 512], mybir.dt.bfloat16, tag="osbA", name="osbA")
                if q == NQ - 1:
                    nc.scalar.activation(
                        osbA[:], psA[:],
                        mybir.ActivationFunctionType.Identity,
                        bias=bcol, scale=1.0 / SCALE,
                    )
                else:
                    nc.vector.tensor_scalar(
                        osbA[:], psA[:],
                        1.0 / SCALE, bcol,
                        mybir.AluOpType.mult, mybir.AluOpType.add,
                    )
                nc.scalar.dma_start(out=outT[osl, c0 : c0 + 512], in_=osbA[:])
                osbB = op.tile([P, 512], mybir.dt.bfloat16, tag="osbB", name="osbB")
                nc.vector.tensor_scalar(
                    osbB[:], psB[:],
                    1.0 / SCALE, bcol,
                    mybir.AluOpType.mult, mybir.AluOpType.add,
                )
                nc.sync.dma_start(out=outT[osl, c0 + 512 : c0 + QCOL], in_=osbB[:])

            # Phase 1 — groups (q0, o=0..3), DR-first. q0 is deep-fp8: each
            # group opens with 8 DR matmuls (pairs 4,5 and 6,7). The psA
            # sides run first across the groups (they need only the A
            # halves of the fp8 data), then psB; then the bf16 part runs
            # k-outer, matching DMA arrival, with the final k-step staggered
            # per group for bubble-free PSUM recycling into phase 2.
            ph1 = []
            for o in range(4):
                psA, psB = ps_first if o == 0 else (
                    pp.tile([P, 512], mybir.dt.float32, tag="pa", name="psA"),
                    pp.tile([P, 512], mybir.dt.float32, tag="pb", name="psB"),
                )
                ph1.append((psA, psB))
            for half in range(2):
                dh = tiles["d8q0A"] if half == 0 else tiles["d8q0B"]
                for gi in range(2):
                    for o in range(4):
                        ps = ph1[o][half]
                        wsl = (tiles["w8lo"][:, :, o * P : (o + 1) * P]
                               if gi == 0 else whi(o))
                        nc.tensor.matmul(
                            ps[:, 0:256], wsl, dh[:, 2 * gi : 2 * gi + 2, 0:256],
                            start=(gi == 0), stop=False, perf_mode=dr,
                            skip_group_check=True,
                        )
                        nc.tensor.matmul(
                            ps[:, 256:512], wsl, dh[:, 2 * gi : 2 * gi + 2, 256:512],
                            start=False, stop=False, perf_mode=dr,
                            skip_group_check=True,
                        )
            for k in range(3):
                for o in range(4):
                    psA, psB = ph1[o]
                    lhsT = w_t[k][:, o * P : (o + 1) * P]
                    nc.tensor.matmul(
                        psA[:], lhsT, d_t[k][0][:, 0:512],
                        start=False, stop=False,
                    )
                    nc.tensor.matmul(
                        psB[:], lhsT, d_t[k][0][:, 512:QCOL],
                        start=False, stop=False,
                    )
            for o in range(4):
                psA, psB = ph1[o]
                lhsT = w_t[3][:, o * P : (o + 1) * P]
                nc.tensor.matmul(
                    psA[:], lhsT, d_t[3][0][:, 0:512],
                    start=False, stop=True,
                )
                nc.tensor.matmul(
                    psB[:], lhsT, d_t[3][0][:, 512:QCOL],
                    start=False, stop=True,
                )
                emit_evac(psA, psB, 0, o)

            # Phase 2 — everything else in normal order (bf16 k-major, DR
            # tail) since all operands are SBUF-resident by then.
            for q in range(NQ):
                for o in range(4 if q == 0 else 0, NO):
                    psA = pp.tile([P, 512], mybir.dt.float32, tag="pa", name="psA")
                    psB = pp.tile([P, 512], mybir.dt.float32, tag="pb", name="psB")
                    for k in range(kmax(q)):
                        lhsT = w_t[k][:, o * P : (o + 1) * P]
                        nc.tensor.matmul(
                            psA[:], lhsT, d_t[k][q][:, 0:512],
                            start=(k == 0), stop=False,
                        )
                        nc.tensor.matmul(
                            psB[:], lhsT, d_t[k][q][:, 512:QCOL],
                            start=(k == 0), stop=False,
                        )
                    emit_dr(psA, psB, q, o, first=False)
                    emit_evac(psA, psB, q, o)

    nc.compile()
    return nc


def _get_nc():
    if "nc" not in _CACHE:
        _CACHE["nc"] = _build()
    return _CACHE["nc"]


def _prep_weights(W, b):
    W = np.asarray(W, dtype=np.float32)
    b = np.asarray(b, dtype=np.float32)
    Ws = W * SCALE
    # wT[k, p, o] = W[o, k*128+p] * 128  (bf16)
    wT = np.ascontiguousarray(
        Ws[:, : KB * P].T.reshape(KB, P, OUT_DIM).astype(BF)
    )
    # w8lo[p, i, o] = e4m3(W[o, 512 + i*128 + p] * 128)  (k-blocks 4,5)
    # w8hi[p, i, o] = e4m3(W[o, 768 + i*128 + p] * 128)  (k-blocks 6,7)
    w8lo = np.ascontiguousarray(
        Ws[:, 4 * P : 6 * P].T.reshape(2, P, OUT_DIM).transpose(1, 0, 2).astype(E4)
    )
    w8hi = np.ascontiguousarray(
        Ws[:, 6 * P :].T.reshape(2, P, OUT_DIM).transpose(1, 0, 2).astype(E4)
    )
    bias2 = np.ascontiguousarray(b.reshape(NO, P).T)  # [128, 8] f32
    return wT, w8lo, w8hi, bias2


def _prep_inputs(data, W, b):
    data = np.asarray(data, dtype=np.float32)
    wT, w8lo, w8hi, bias2 = _prep_weights(W, b)
    in_maps = []
    for c in range(N_CORES):
        shard = data[c * SHARD : (c + 1) * SHARD]  # [4096, 1024] f32
        # dT[k, p, b] = bf16(shard[b, k*128+p])
        dTc = np.ascontiguousarray(
            shard[:, : KB * P].T.reshape(KB, P, SHARD).astype(BF)
        )
        # d8a: shallow quarters q1, q2 (rows 1024:3072), k-blocks 6,7
        d8at = shard[QCOL : 3 * QCOL, 6 * P :].T.reshape(2, P, 2, QCOL)
        d8ac = np.ascontiguousarray(d8at.transpose(2, 1, 0, 3).astype(E4))
        # d8b: deep quarters q0, q3 (rows 0:1024 and 3072:4096), k-blocks
        # 4..7
        deep_rows = np.concatenate(
            [shard[:QCOL, 4 * P :], shard[3 * QCOL :, 4 * P :]]
        )
        d8bt = deep_rows.T.reshape(4, P, 2, QCOL)
        d8bc = np.ascontiguousarray(d8bt.transpose(2, 1, 0, 3).astype(E4))
        in_maps.append(
            {"dT": dTc, "d8a": d8ac, "d8b": d8bc, "wT": wT, "w8lo": w8lo,
             "w8hi": w8hi, "biasb": bias2}
        )
    return in_maps


def _run(data, W, b, trace=False, **trace_kw):
    nc = _get_nc()
    in_maps = _prep_inputs(data, W, b)
    res = run_bass_kernel_spmd(
        nc, in_maps, list(range(N_CORES)), trace=trace, **trace_kw
    )
    out = np.concatenate(
        [
            np.asarray(res.results[c]["outT"]).T.astype(np.float32)
            for c in range(N_CORES)
        ],
        axis=0,
    )
    return out, res


def kernel(**inputs) -> np.ndarray:
    out, _ = _run(inputs["data"], inputs["W"], inputs["b"])
    return out


# revision 26
# speedup vs baseline: 1.0208x; 1.0208x over previous
"""GroupFC kernel for Trainium2, data-parallel across 8 NeuronCores.

Problem: out = data @ W.T + b
  data: [32768, 1024] f32, W: [1024, 1024] f32, b: [1024] f32

Strategy:
  - Shard batch dim across 8 cores (4096 rows each); replicate W, b.
  - Transposed-output formulation: outT[o, b] = sum_k W[o,k] d[b,k] + b[o].
    Stationary operand = W tiles (out-dim on PSUM partitions), moving
    operand = data columns (batch on the free dim).
  - Mixed precision along the contraction, tuned to the 2e-2 rel-err
    budget: two batch quarters (q1, q2) run k-blocks 0..5 in bf16
    (1 col/cycle) and blocks 6,7 in fp8-e4m3 DoubleRow (measured ~2x
    column rate); the other two (q0, q3) run blocks 0..3 bf16 and 4..7
    fp8 (two DR pairs). Measured rel err ~1.87e-2.
  - q0 is a deep-fp8 quarter on purpose: the DR-first ramp phase gets
    ~2x the fp8 work from ~1 MiB of loads, so the PE is busy while the
    bf16 tiles stream in, and the bf16 k-outer ramp is 4 steps, not 6.
  - All W values pre-scaled by 128 on the host so the fp8 weights avoid
    the e4m3 subnormal range; the fused evacuation applies 1/128 and the
    per-out-row bias in one pass per bank (DVE; ACT joins for the last
    quarter), emitting bf16 halves stored immediately on both HWDGE
    rings.
  - Startup: memset-gated warmup matmuls ramp the PE HAM clock gate.
  - Host post-pass transposes outT back to [batch, out] f32.
"""

import sys
from contextlib import ExitStack

import numpy as np

try:
    import concourse.bass as bass  # noqa: F401
except ImportError:
    sys.path.insert(0, "/opt/trn_rl_repo")

import ml_dtypes

import concourse.tile as tile
from concourse import bacc, mybir
from concourse.bass_utils import run_bass_kernel_spmd

N_CORES = 8
BATCH = 32768
SHARD = BATCH // N_CORES  # 4096
IN_DIM = 1024
OUT_DIM = 1024
P = 128
KB = 6  # bf16 k-blocks for shallow quarters; deep quarters use KB-2
NQ = 4  # batch quarters per core (1024 columns each)
QCOL = SHARD // NQ  # 1024
NO = OUT_DIM // P  # 8 output-row blocks
SCALE = 128.0
DEEP = (True, False, False, True)  # per-quarter: 4 fp8 k-blocks vs 2
E4 = ml_dtypes.float8_e4m3
BF = ml_dtypes.bfloat16

_CACHE = {}


def _build():
    nc = bacc.Bacc("TRN2", target_bir_lowering=False, debug=False)
    dT = nc.dram_tensor(
        "dT", [KB, P, SHARD], mybir.dt.bfloat16, kind="ExternalInput"
    ).ap()
    d8a = nc.dram_tensor(  # shallow quarters (q1, q2), k-blocks 6,7
        "d8a", [2, P, 2, QCOL], mybir.dt.float8e4, kind="ExternalInput"
    ).ap()
    d8b = nc.dram_tensor(  # deep quarters (q0, q3), k-blocks 4..7
        "d8b", [2, P, 4, QCOL], mybir.dt.float8e4, kind="ExternalInput"
    ).ap()
    d8c = nc.dram_tensor(  # extra deep groups (q2, o>=6), k-blocks 4,5
        "d8c", [P, 2, QCOL], mybir.dt.float8e4, kind="ExternalInput"
    ).ap()
    wT = nc.dram_tensor(
        "wT", [KB, P, OUT_DIM], mybir.dt.bfloat16, kind="ExternalInput"
    ).ap()
    w8hi = nc.dram_tensor(  # fp8 weights, k-blocks 6,7
        "w8hi", [P, 2, OUT_DIM], mybir.dt.float8e4, kind="ExternalInput"
    ).ap()
    w8lo = nc.dram_tensor(  # fp8 weights, k-blocks 4,5
        "w8lo", [P, 2, OUT_DIM], mybir.dt.float8e4, kind="ExternalInput"
    ).ap()
    biasb = nc.dram_tensor(
        "biasb", [P, NO], mybir.dt.float32, kind="ExternalInput"
    ).ap()
    outT = nc.dram_tensor(
        "outT", [OUT_DIM, SHARD], mybir.dt.bfloat16, kind="ExternalOutput"
    ).ap()

    with tile.TileContext(nc) as tc:
        with ExitStack() as ctx:
            wp = ctx.enter_context(tc.tile_pool(name="w", bufs=1))
            dp = ctx.enter_context(tc.tile_pool(name="d", bufs=1))
            bp = ctx.enter_context(tc.tile_pool(name="misc", bufs=1))
            pp = ctx.enter_context(tc.tile_pool(name="psum", bufs=4, space="PSUM"))
            op = ctx.enter_context(tc.tile_pool(name="o", bufs=8))

            w_t = [None] * KB
            d_t = [[None] * NQ for _ in range(KB)]
            d8_t = [None] * NQ  # q0 uses the half tiles below instead
            tiles = {}

            def kmax(q):
                return KB - 2 if DEEP[q] else KB

            # Load plan. Critical ramp first: the fp8 weights (both pairs)
            # and the q0 fp8 data halves unlock the DR-first phase from
            # ~1.1 MiB; then (wT[k], dT[k] q0) pairs k=0..3 in consumption
            # order; then the rest. Alternate the two HWDGE rings.
            loads = [
                ("bias", 0, 0), ("w8hiA", 0, 0), ("d8q0A", 0, 0),
                ("w8lo", 0, 0), ("d8q0B", 0, 0), ("w8hiB", 0, 0),
            ]
            for k in range(4):
                loads.append(("w", k, 0))
                loads.append(("d", k, 0))
            loads += [("w", 4, 0), ("w", 5, 0)]
            for q in range(1, NQ):
                for k in range(kmax(q)):
                    loads.append(("d", k, q))
                loads.append(("d8", 0, q))
                if q == 2:
                    loads.append(("d8c", 0, 0))

            for i, (kind, k, q) in enumerate(loads):
                eng = nc.scalar if i % 2 == 0 else nc.sync
                if kind == "w":
                    w_t[k] = wp.tile([P, OUT_DIM], mybir.dt.bfloat16, tag=f"w{k}", name=f"w_t{k}")
                    eng.dma_start(out=w_t[k][:], in_=wT[k, :, :])
                elif kind == "d":
                    d_t[k][q] = dp.tile([P, QCOL], mybir.dt.bfloat16, tag=f"d{k}_{q}", name=f"d_t{k}_{q}")
                    eng.dma_start(
                        out=d_t[k][q][:], in_=dT[k, :, q * QCOL : (q + 1) * QCOL]
                    )
                elif kind == "w8hiA":
                    tiles["w8hiA"] = wp.tile([P, 2, 512], mybir.dt.float8e4, tag="w8hiA", name="w8hiA_t")
                    eng.dma_start(out=tiles["w8hiA"][:], in_=w8hi[:, :, 0:512])
                elif kind == "w8hiB":
                    tiles["w8hiB"] = wp.tile([P, 2, 512], mybir.dt.float8e4, tag="w8hiB", name="w8hiB_t")
                    eng.dma_start(out=tiles["w8hiB"][:], in_=w8hi[:, :, 512:OUT_DIM])
                elif kind == "w8lo":
                    tiles["w8lo"] = wp.tile([P, 2, OUT_DIM], mybir.dt.float8e4, tag="w8lo", name="w8lo_t")
                    eng.dma_start(out=tiles["w8lo"][:], in_=w8lo[:, :, :])
                elif kind == "d8q0A":
                    tiles["d8q0A"] = dp.tile([P, 4, 512], mybir.dt.float8e4, tag="d8q0A", name="d8q0A_t")
                    eng.dma_start(out=tiles["d8q0A"][:], in_=d8b[0, :, :, 0:512])
                elif kind == "d8q0B":
                    tiles["d8q0B"] = dp.tile([P, 4, 512], mybir.dt.float8e4, tag="d8q0B", name="d8q0B_t")
                    eng.dma_start(out=tiles["d8q0B"][:], in_=d8b[0, :, :, 512:QCOL])
                elif kind == "d8c":
                    tiles["d8c"] = dp.tile([P, 2, QCOL], mybir.dt.float8e4, tag="d8c", name="d8c_t")
                    eng.dma_start(out=tiles["d8c"][:], in_=d8c[:, :, :])
                elif kind == "d8":
                    nblk = 4 if DEEP[q] else 2
                    d8_t[q] = dp.tile([P, nblk, QCOL], mybir.dt.float8e4, tag=f"d8_{q}", name=f"d8_t{q}")
                    src = d8b[1] if DEEP[q] else d8a[q - 1]
                    eng.dma_start(out=d8_t[q][:], in_=src[:, :, :])
                else:
                    bias_t = bp.tile([P, NO], mybir.dt.float32, tag="bias", name="bias_t")
                    eng.dma_start(out=bias_t[:], in_=biasb[:, :])

            # Warmup: ramp the PE HAM clock while loads stream. Gated on an
            # on-chip memset so it starts as soon as the engines come up.
            scr = bp.tile([P, 256], mybir.dt.bfloat16, tag="scr", name="scr")
            nc.vector.memset(scr[:], 0)
            ps_first = [
                pp.tile([P, 512], mybir.dt.float32, tag="pa", name="ps_a0"),
                pp.tile([P, 512], mybir.dt.float32, tag="pb", name="ps_b0"),
            ]
            for i in range(24):
                nc.tensor.matmul(
                    ps_first[0][:, 0:256], scr[:, 0:P], scr[:],
                    start=True, stop=True, skip_group_check=True,
                )

            dr = mybir.MatmulPerfMode.DoubleRow

            def whi(o):
                t = tiles["w8hiA"] if o < 4 else tiles["w8hiB"]
                return t[:, :, (o % 4) * P : (o % 4 + 1) * P]

            def emit_dr(psA, psB, q, o, first, xdeep=False):
                # Only the FIRST matmul per bank may set start=True: start
                # clears has_written for the WHOLE bank, so a second start on
                # the other half would wipe the first half's result.
                osl = slice(o * P, (o + 1) * P)
                npair = 2 if (DEEP[q] or xdeep) else 1
                for gi in range(npair):
                    if DEEP[q]:
                        wsl = tiles["w8lo"][:, :, osl] if gi == 0 else whi(o)
                        dlo = 2 * gi
                    elif xdeep:
                        wsl = tiles["w8lo"][:, :, osl] if gi == 0 else whi(o)
                        dlo = 0
                    else:
                        wsl = whi(o)
                        dlo = 0
                    if xdeep and gi == 0:
                        dsl = tiles["d8c"]
                        srcs = [
                            dsl[:, 0:2, 0:256],
                            dsl[:, 0:2, 256:512],
                            dsl[:, 0:2, 512:768],
                            dsl[:, 0:2, 768:QCOL],
                        ]
                    elif q == 0:
                        srcs = [
                            tiles["d8q0A"][:, dlo : dlo + 2, 0:256],
                            tiles["d8q0A"][:, dlo : dlo + 2, 256:512],
                            tiles["d8q0B"][:, dlo : dlo + 2, 0:256],
                            tiles["d8q0B"][:, dlo : dlo + 2, 256:512],
                        ]
                    else:
                        dsl = d8_t[q]
                        srcs = [
                            dsl[:, dlo : dlo + 2, 0:256],
                            dsl[:, dlo : dlo + 2, 256:512],
                            dsl[:, dlo : dlo + 2, 512:768],
                            dsl[:, dlo : dlo + 2, 768:QCOL],
                        ]
                    last = gi == npair - 1
                    st = first and gi == 0
                    nc.tensor.matmul(
                        psA[:, 0:256], wsl, srcs[0],
                        start=st, stop=(not first) and last, perf_mode=dr,
                        skip_group_check=True,
                    )
                    nc.tensor.matmul(
                        psA[:, 256:512], wsl, srcs[1],
                        start=False, stop=(not first) and last, perf_mode=dr,
                        skip_group_check=True,
                    )
                    nc.tensor.matmul(
                        psB[:, 0:256], wsl, srcs[2],
                        start=st, stop=(not first) and last, perf_mode=dr,
                        skip_group_check=True,
                    )
                    nc.tensor.matmul(
                        psB[:, 256:512], wsl, srcs[3],
                        start=False, stop=(not first) and last, perf_mode=dr,
                        skip_group_check=True,
                    )

            def emit_evac(psA, psB, q, o):
                # Fused evacuation: out = psum/128 + bias[o], to bf16, with
                # separate half-tiles so each store only waits on its own
                # evacuation and rides its own HW ring. Evacs stay on DVE
                # (the ring queues carry the load DMAs); the last quarter
                # also uses ACT, whose queue has drained by then.
                osl = slice(o * P, (o + 1) * P)
                bcol = bias_t[:, o : o + 1]
                c0 = q * QCOL
                osbA = op.tile([P, 512], mybir.dt.bfloat16, tag="osbA", name="osbA")
                if q == NQ - 1:
                    nc.scalar.activation(
                        osbA[:], psA[:],
                        mybir.ActivationFunctionType.Identity,
                        bias=bcol, scale=1.0 / SCALE,
                    )
                else:
                    nc.vector.tensor_scalar(
                        osbA[:], psA[:],
                        1.0 / SCALE, bcol,
                        mybir.AluOpType.mult, mybir.AluOpType.add,
                    )
                nc.scalar.dma_start(out=outT[osl, c0 : c0 + 512], in_=osbA[:])
                osbB = op.tile([P, 512], mybir.dt.bfloat16, tag="osbB", name="osbB")
                nc.vector.tensor_scalar(
                    osbB[:], psB[:],
                    1.0 / SCALE, bcol,
                    mybir.AluOpType.mult, mybir.AluOpType.add,
                )
                nc.sync.dma_start(out=outT[osl, c0 + 512 : c0 + QCOL], in_=osbB[:])

            # Phase 1 — groups (q0, o=0..3), DR-first. q0 is deep-fp8: each
            # group opens with 8 DR matmuls (pairs 4,5 and 6,7). The psA
            # sides run first across the groups (they need only the A
            # halves of the fp8 data), then psB; then the bf16 part runs
            # k-outer, matching DMA arrival, with the final k-step staggered
            # per group for bubble-free PSUM recycling into phase 2.
            ph1 = []
            for o in range(4):
                psA, psB = ps_first if o == 0 else (
                    pp.tile([P, 512], mybir.dt.float32, tag="pa", name="psA"),
                    pp.tile([P, 512], mybir.dt.float32, tag="pb", name="psB"),
                )
                ph1.append((psA, psB))
            for half in range(2):
                dh = tiles["d8q0A"] if half == 0 else tiles["d8q0B"]
                for gi in range(2):
                    for o in range(4):
                        ps = ph1[o][half]
                        wsl = (tiles["w8lo"][:, :, o * P : (o + 1) * P]
                               if gi == 0 else whi(o))
                        nc.tensor.matmul(
                            ps[:, 0:256], wsl, dh[:, 2 * gi : 2 * gi + 2, 0:256],
                            start=(gi == 0), stop=False, perf_mode=dr,
                            skip_group_check=True,
                        )
                        nc.tensor.matmul(
                            ps[:, 256:512], wsl, dh[:, 2 * gi : 2 * gi + 2, 256:512],
                            start=False, stop=False, perf_mode=dr,
                            skip_group_check=True,
                        )
            for k in range(3):
                for o in range(4):
                    psA, psB = ph1[o]
                    lhsT = w_t[k][:, o * P : (o + 1) * P]
                    nc.tensor.matmul(
                        psA[:], lhsT, d_t[k][0][:, 0:512],
                        start=False, stop=False,
                    )
                    nc.tensor.matmul(
                        psB[:], lhsT, d_t[k][0][:, 512:QCOL],
                        start=False, stop=False,
                    )
            for o in range(4):
                psA, psB = ph1[o]
                lhsT = w_t[3][:, o * P : (o + 1) * P]
                nc.tensor.matmul(
                    psA[:], lhsT, d_t[3][0][:, 0:512],
                    start=False, stop=True,
                )
                nc.tensor.matmul(
                    psB[:], lhsT, d_t[3][0][:, 512:QCOL],
                    start=False, stop=True,
                )
                emit_evac(psA, psB, 0, o)

            # Phase 2 — everything else in normal order (bf16 k-major, DR
            # tail) since all operands are SBUF-resident by then.
            for q in range(NQ):
                for o in range(4 if q == 0 else 0, NO):
                    psA = pp.tile([P, 512], mybir.dt.float32, tag="pa", name="psA")
                    psB = pp.tile([P, 512], mybir.dt.float32, tag="pb", name="psB")
                    xdeep = q == 2 and o >= 6
                    for k in range(4 if xdeep else kmax(q)):
                        lhsT = w_t[k][:, o * P : (o + 1) * P]
                        nc.tensor.matmul(
                            psA[:], lhsT, d_t[k][q][:, 0:512],
                            start=(k == 0), stop=False,
                        )
                        nc.tensor.matmul(
                            psB[:], lhsT, d_t[k][q][:, 512:QCOL],
                            start=(k == 0), stop=False,
                        )
                    emit_dr(psA, psB, q, o, first=False, xdeep=xdeep)
                    emit_evac(psA, psB, q, o)

    nc.compile()
    return nc


def _get_nc():
    if "nc" not in _CACHE:
        _CACHE["nc"] = _build()
    return _CACHE["nc"]


def _prep_weights(W, b):
    W = np.asarray(W, dtype=np.float32)
    b = np.asarray(b, dtype=np.float32)
    Ws = W * SCALE
    # wT[k, p, o] = W[o, k*128+p] * 128  (bf16)
    wT = np.ascontiguousarray(
        Ws[:, : KB * P].T.reshape(KB, P, OUT_DIM).astype(BF)
    )
    # w8lo[p, i, o] = e4m3(W[o, 512 + i*128 + p] * 128)  (k-blocks 4,5)
    # w8hi[p, i, o] = e4m3(W[o, 768 + i*128 + p] * 128)  (k-blocks 6,7)
    w8lo = np.ascontiguousarray(
        Ws[:, 4 * P : 6 * P].T.reshape(2, P, OUT_DIM).transpose(1, 0, 2).astype(E4)
    )
    w8hi = np.ascontiguousarray(
        Ws[:, 6 * P :].T.reshape(2, P, OUT_DIM).transpose(1, 0, 2).astype(E4)
    )
    bias2 = np.ascontiguousarray(b.reshape(NO, P).T)  # [128, 8] f32
    return wT, w8lo, w8hi, bias2


def _prep_inputs(data, W, b):
    data = np.asarray(data, dtype=np.float32)
    wT, w8lo, w8hi, bias2 = _prep_weights(W, b)
    in_maps = []
    for c in range(N_CORES):
        shard = data[c * SHARD : (c + 1) * SHARD]  # [4096, 1024] f32
        # dT[k, p, b] = bf16(shard[b, k*128+p])
        dTc = np.ascontiguousarray(
            shard[:, : KB * P].T.reshape(KB, P, SHARD).astype(BF)
        )
        # d8a: shallow quarters q1, q2 (rows 1024:3072), k-blocks 6,7
        d8at = shard[QCOL : 3 * QCOL, 6 * P :].T.reshape(2, P, 2, QCOL)
        d8ac = np.ascontiguousarray(d8at.transpose(2, 1, 0, 3).astype(E4))
        # d8b: deep quarters q0, q3 (rows 0:1024 and 3072:4096), k-blocks
        # 4..7
        deep_rows = np.concatenate(
            [shard[:QCOL, 4 * P :], shard[3 * QCOL :, 4 * P :]]
        )
        d8bt = deep_rows.T.reshape(4, P, 2, QCOL)
        d8bc = np.ascontiguousarray(d8bt.transpose(2, 1, 0, 3).astype(E4))
        # d8c: q2 rows (2048:3072), k-blocks 4,5 -- extra deep groups o>=6
        d8ct = shard[2 * QCOL : 3 * QCOL, 4 * P : 6 * P].T.reshape(2, P, QCOL)
        d8cc = np.ascontiguousarray(d8ct.transpose(1, 0, 2).astype(E4))
        in_maps.append(
            {"dT": dTc, "d8a": d8ac, "d8b": d8bc, "d8c": d8cc, "wT": wT,
             "w8lo": w8lo, "w8hi": w8hi, "biasb": bias2}
        )
    return in_maps


def _run(data, W, b, trace=False, **trace_kw):
    nc = _get_nc()
    in_maps = _prep_inputs(data, W, b)
    res = run_bass_kernel_spmd(
        nc, in_maps, list(range(N_CORES)), trace=trace, **trace_kw
    )
    out = np.concatenate(
        [
            np.asarray(res.results[c]["outT"]).T.astype(np.float32)
            for c in range(N_CORES)
        ],
        axis=0,
    )
    return out, res


def kernel(**inputs) -> np.ndarray:
    out, _ = _run(inputs["data"], inputs["W"], inputs["b"])
    return out


# revision 27
# speedup vs baseline: 1.0349x; 1.0138x over previous
"""GroupFC kernel for Trainium2, data-parallel across 8 NeuronCores.

Problem: out = data @ W.T + b
  data: [32768, 1024] f32, W: [1024, 1024] f32, b: [1024] f32

Strategy:
  - Shard batch dim across 8 cores (4096 rows each); replicate W, b.
  - Transposed-output formulation: outT[o, b] = sum_k W[o,k] d[b,k] + b[o].
    Stationary operand = W tiles (out-dim on PSUM partitions), moving
    operand = data columns (batch on the free dim).
  - Mixed precision along the contraction, tuned to the 2e-2 rel-err
    budget: two batch quarters (q1, q2) run k-blocks 0..5 in bf16
    (1 col/cycle) and blocks 6,7 in fp8-e4m3 DoubleRow (measured ~2x
    column rate); the other two (q0, q3) run blocks 0..3 bf16 and 4..7
    fp8 (two DR pairs). Measured rel err ~1.87e-2.
  - q0 is a deep-fp8 quarter on purpose: the DR-first ramp phase gets
    ~2x the fp8 work from ~1 MiB of loads, so the PE is busy while the
    bf16 tiles stream in, and the bf16 k-outer ramp is 4 steps, not 6.
  - All W values pre-scaled by 128 on the host so the fp8 weights avoid
    the e4m3 subnormal range; the fused evacuation applies 1/128 and the
    per-out-row bias in one pass per bank (DVE; ACT joins for the last
    quarter), emitting bf16 halves stored immediately on both HWDGE
    rings.
  - Startup: memset-gated warmup matmuls ramp the PE HAM clock gate.
  - Host post-pass transposes outT back to [batch, out] f32.
"""

import sys
from contextlib import ExitStack

import numpy as np

try:
    import concourse.bass as bass  # noqa: F401
except ImportError:
    sys.path.insert(0, "/opt/trn_rl_repo")

import ml_dtypes

import concourse.tile as tile
from concourse import bacc, mybir
from concourse.bass_utils import run_bass_kernel_spmd

N_CORES = 8
BATCH = 32768
SHARD = BATCH // N_CORES  # 4096
IN_DIM = 1024
OUT_DIM = 1024
P = 128
KB = 6  # bf16 k-blocks for shallow quarters; deep quarters use KB-2
NQ = 4  # batch quarters per core (1024 columns each)
QCOL = SHARD // NQ  # 1024
NO = OUT_DIM // P  # 8 output-row blocks
SCALE = 128.0
DEEP = (True, False, False, True)  # per-quarter: 4 fp8 k-blocks vs 2
E4 = ml_dtypes.float8_e4m3
BF = ml_dtypes.bfloat16

_CACHE = {}


def _build():
    nc = bacc.Bacc("TRN2", target_bir_lowering=False, debug=False)
    dT = nc.dram_tensor(
        "dT", [KB, P, SHARD], mybir.dt.bfloat16, kind="ExternalInput"
    ).ap()
    d8a = nc.dram_tensor(  # shallow quarters (q1, q2), k-blocks 6,7
        "d8a", [2, P, 2, QCOL], mybir.dt.float8e4, kind="ExternalInput"
    ).ap()
    d8b = nc.dram_tensor(  # deep quarters (q0, q3), k-blocks 4..7
        "d8b", [2, P, 4, QCOL], mybir.dt.float8e4, kind="ExternalInput"
    ).ap()
    d8c = nc.dram_tensor(  # extra deep groups (q2, o>=6), k-blocks 4,5
        "d8c", [P, 2, QCOL], mybir.dt.float8e4, kind="ExternalInput"
    ).ap()
    wT = nc.dram_tensor(
        "wT", [KB, P, OUT_DIM], mybir.dt.bfloat16, kind="ExternalInput"
    ).ap()
    w8hi = nc.dram_tensor(  # fp8 weights, k-blocks 6,7
        "w8hi", [P, 2, OUT_DIM], mybir.dt.float8e4, kind="ExternalInput"
    ).ap()
    w8lo = nc.dram_tensor(  # fp8 weights, k-blocks 4,5
        "w8lo", [P, 2, OUT_DIM], mybir.dt.float8e4, kind="ExternalInput"
    ).ap()
    biasb = nc.dram_tensor(
        "biasb", [P, NO], mybir.dt.float32, kind="ExternalInput"
    ).ap()
    outT = nc.dram_tensor(
        "outT", [OUT_DIM, SHARD], mybir.dt.bfloat16, kind="ExternalOutput"
    ).ap()

    with tile.TileContext(nc) as tc:
        with ExitStack() as ctx:
            wp = ctx.enter_context(tc.tile_pool(name="w", bufs=1))
            dp = ctx.enter_context(tc.tile_pool(name="d", bufs=1))
            bp = ctx.enter_context(tc.tile_pool(name="misc", bufs=1))
            pp = ctx.enter_context(tc.tile_pool(name="psum", bufs=4, space="PSUM"))
            op = ctx.enter_context(tc.tile_pool(name="o", bufs=8))

            w_t = [None] * KB
            d_t = [[None] * NQ for _ in range(KB)]
            d8_t = [None] * NQ  # q0 uses the half tiles below instead
            tiles = {}

            def kmax(q):
                return KB - 2 if DEEP[q] else KB

            # Load plan. Critical ramp first: the fp8 weights (both pairs)
            # and the q0 fp8 data halves unlock the DR-first phase from
            # ~1.1 MiB; then (wT[k], dT[k] q0) pairs k=0..3 in consumption
            # order; then the rest. Alternate the two HWDGE rings.
            loads = [
                ("bias", 0, 0), ("w8hiA", 0, 0), ("d8q0A", 0, 0),
                ("w8lo", 0, 0), ("d8q0B", 0, 0), ("w8hiB", 0, 0),
            ]
            for k in range(4):
                loads.append(("w", k, 0))
                loads.append(("d", k, 0))
            loads += [("w", 4, 0), ("w", 5, 0)]
            for q in range(1, NQ):
                for k in range(kmax(q)):
                    loads.append(("d", k, q))
                loads.append(("d8", 0, q))
                if q == 2:
                    loads.append(("d8c", 0, 0))

            for i, (kind, k, q) in enumerate(loads):
                eng = nc.scalar if i % 2 == 0 else nc.sync
                if kind == "w":
                    w_t[k] = wp.tile([P, OUT_DIM], mybir.dt.bfloat16, tag=f"w{k}", name=f"w_t{k}")
                    eng.dma_start(out=w_t[k][:], in_=wT[k, :, :])
                elif kind == "d":
                    d_t[k][q] = dp.tile([P, QCOL], mybir.dt.bfloat16, tag=f"d{k}_{q}", name=f"d_t{k}_{q}")
                    eng.dma_start(
                        out=d_t[k][q][:], in_=dT[k, :, q * QCOL : (q + 1) * QCOL]
                    )
                elif kind == "w8hiA":
                    tiles["w8hiA"] = wp.tile([P, 2, 512], mybir.dt.float8e4, tag="w8hiA", name="w8hiA_t")
                    eng.dma_start(out=tiles["w8hiA"][:], in_=w8hi[:, :, 0:512])
                elif kind == "w8hiB":
                    tiles["w8hiB"] = wp.tile([P, 2, 512], mybir.dt.float8e4, tag="w8hiB", name="w8hiB_t")
                    eng.dma_start(out=tiles["w8hiB"][:], in_=w8hi[:, :, 512:OUT_DIM])
                elif kind == "w8lo":
                    tiles["w8lo"] = wp.tile([P, 2, OUT_DIM], mybir.dt.float8e4, tag="w8lo", name="w8lo_t")
                    eng.dma_start(out=tiles["w8lo"][:], in_=w8lo[:, :, :])
                elif kind == "d8q0A":
                    tiles["d8q0A"] = dp.tile([P, 4, 512], mybir.dt.float8e4, tag="d8q0A", name="d8q0A_t")
                    eng.dma_start(out=tiles["d8q0A"][:], in_=d8b[0, :, :, 0:512])
                elif kind == "d8q0B":
                    tiles["d8q0B"] = dp.tile([P, 4, 512], mybir.dt.float8e4, tag="d8q0B", name="d8q0B_t")
                    eng.dma_start(out=tiles["d8q0B"][:], in_=d8b[0, :, :, 512:QCOL])
                elif kind == "d8c":
                    tiles["d8c"] = dp.tile([P, 2, QCOL], mybir.dt.float8e4, tag="d8c", name="d8c_t")
                    eng.dma_start(out=tiles["d8c"][:], in_=d8c[:, :, :])
                elif kind == "d8":
                    nblk = 4 if DEEP[q] else 2
                    d8_t[q] = dp.tile([P, nblk, QCOL], mybir.dt.float8e4, tag=f"d8_{q}", name=f"d8_t{q}")
                    src = d8b[1] if DEEP[q] else d8a[q - 1]
                    eng.dma_start(out=d8_t[q][:], in_=src[:, :, :])
                else:
                    bias_t = bp.tile([P, NO], mybir.dt.float32, tag="bias", name="bias_t")
                    eng.dma_start(out=bias_t[:], in_=biasb[:, :])

            # Warmup: ramp the PE HAM clock while loads stream. Gated on an
            # on-chip memset so it starts as soon as the engines come up.
            scr = bp.tile([P, 256], mybir.dt.bfloat16, tag="scr", name="scr")
            nc.vector.memset(scr[:], 0)
            ps_first = [
                pp.tile([P, 512], mybir.dt.float32, tag="pa", name="ps_a0"),
                pp.tile([P, 512], mybir.dt.float32, tag="pb", name="ps_b0"),
            ]
            for i in range(24):
                nc.tensor.matmul(
                    ps_first[0][:, 0:256], scr[:, 0:P], scr[:],
                    start=True, stop=True, skip_group_check=True,
                )

            dr = mybir.MatmulPerfMode.DoubleRow

            def whi(o):
                t = tiles["w8hiA"] if o < 4 else tiles["w8hiB"]
                return t[:, :, (o % 4) * P : (o % 4 + 1) * P]

            def emit_dr(psA, psB, q, o, first, xdeep=False):
                # Only the FIRST matmul per bank may set start=True: start
                # clears has_written for the WHOLE bank, so a second start on
                # the other half would wipe the first half's result.
                osl = slice(o * P, (o + 1) * P)
                npair = 2 if (DEEP[q] or xdeep) else 1
                for gi in range(npair):
                    if DEEP[q]:
                        wsl = tiles["w8lo"][:, :, osl] if gi == 0 else whi(o)
                        dlo = 2 * gi
                    elif xdeep:
                        wsl = tiles["w8lo"][:, :, osl] if gi == 0 else whi(o)
                        dlo = 0
                    else:
                        wsl = whi(o)
                        dlo = 0
                    if xdeep and gi == 0:
                        dsl = tiles["d8c"]
                        srcs = [
                            dsl[:, 0:2, 0:256],
                            dsl[:, 0:2, 256:512],
                            dsl[:, 0:2, 512:768],
                            dsl[:, 0:2, 768:QCOL],
                        ]
                    elif q == 0:
                        srcs = [
                            tiles["d8q0A"][:, dlo : dlo + 2, 0:256],
                            tiles["d8q0A"][:, dlo : dlo + 2, 256:512],
                            tiles["d8q0B"][:, dlo : dlo + 2, 0:256],
                            tiles["d8q0B"][:, dlo : dlo + 2, 256:512],
                        ]
                    else:
                        dsl = d8_t[q]
                        srcs = [
                            dsl[:, dlo : dlo + 2, 0:256],
                            dsl[:, dlo : dlo + 2, 256:512],
                            dsl[:, dlo : dlo + 2, 512:768],
                            dsl[:, dlo : dlo + 2, 768:QCOL],
                        ]
                    last = gi == npair - 1
                    st = first and gi == 0
                    nc.tensor.matmul(
                        psA[:, 0:256], wsl, srcs[0],
                        start=st, stop=(not first) and last, perf_mode=dr,
                        skip_group_check=True,
                    )
                    nc.tensor.matmul(
                        psA[:, 256:512], wsl, srcs[1],
                        start=False, stop=(not first) and last, perf_mode=dr,
                        skip_group_check=True,
                    )
                    nc.tensor.matmul(
                        psB[:, 0:256], wsl, srcs[2],
                        start=st, stop=(not first) and last, perf_mode=dr,
                        skip_group_check=True,
                    )
                    nc.tensor.matmul(
                        psB[:, 256:512], wsl, srcs[3],
                        start=False, stop=(not first) and last, perf_mode=dr,
                        skip_group_check=True,
                    )

            def emit_evac(psA, psB, q, o):
                # Fused evacuation: out = psum/128 + bias[o], to bf16, with
                # separate half-tiles so each store only waits on its own
                # evacuation and rides its own HW ring. Evacs stay on DVE
                # (the ring queues carry the load DMAs); the last quarter
                # also uses ACT, whose queue has drained by then.
                osl = slice(o * P, (o + 1) * P)
                bcol = bias_t[:, o : o + 1]
                c0 = q * QCOL
                osbA = op.tile([P, 512], mybir.dt.bfloat16, tag="osbA", name="osbA")
                if q == NQ - 1:
                    nc.scalar.activation(
                        osbA[:], psA[:],
                        mybir.ActivationFunctionType.Identity,
                        bias=bcol, scale=1.0 / SCALE,
                    )
                else:
                    nc.vector.tensor_scalar(
                        osbA[:], psA[:],
                        1.0 / SCALE, bcol,
                        mybir.AluOpType.mult, mybir.AluOpType.add,
                    )
                nc.scalar.dma_start(out=outT[osl, c0 : c0 + 512], in_=osbA[:])
                osbB = op.tile([P, 512], mybir.dt.bfloat16, tag="osbB", name="osbB")
                nc.vector.tensor_scalar(
                    osbB[:], psB[:],
                    1.0 / SCALE, bcol,
                    mybir.AluOpType.mult, mybir.AluOpType.add,
                )
                nc.sync.dma_start(out=outT[osl, c0 + 512 : c0 + QCOL], in_=osbB[:])

            # Phase 1 — groups (q0, o=0..3), DR-first. q0 is deep-fp8: each
            # group opens with 8 DR matmuls (pairs 4,5 and 6,7). The psA
            # sides run first across the groups (they need only the A
            # halves of the fp8 data), then psB; then the bf16 part runs
            # k-outer, matching DMA arrival, with the final k-step staggered
            # per group for bubble-free PSUM recycling into phase 2.
            ph1 = []
            for o in range(4):
                psA, psB = ps_first if o == 0 else (
                    pp.tile([P, 512], mybir.dt.float32, tag="pa", name="psA"),
                    pp.tile([P, 512], mybir.dt.float32, tag="pb", name="psB"),
                )
                ph1.append((psA, psB))
            for half in range(2):
                dh = tiles["d8q0A"] if half == 0 else tiles["d8q0B"]
                for gi in range(2):
                    for o in range(4):
                        ps = ph1[o][half]
                        wsl = (tiles["w8lo"][:, :, o * P : (o + 1) * P]
                               if gi == 0 else whi(o))
                        nc.tensor.matmul(
                            ps[:, 0:256], wsl, dh[:, 2 * gi : 2 * gi + 2, 0:256],
                            start=(gi == 0), stop=False, perf_mode=dr,
                            skip_group_check=True,
                        )
                        nc.tensor.matmul(
                            ps[:, 256:512], wsl, dh[:, 2 * gi : 2 * gi + 2, 256:512],
                            start=False, stop=False, perf_mode=dr,
                            skip_group_check=True,
                        )
            for k in range(3):
                for o in range(4):
                    psA, psB = ph1[o]
                    lhsT = w_t[k][:, o * P : (o + 1) * P]
                    nc.tensor.matmul(
                        psA[:], lhsT, d_t[k][0][:, 0:512],
                        start=False, stop=False,
                    )
                    nc.tensor.matmul(
                        psB[:], lhsT, d_t[k][0][:, 512:QCOL],
                        start=False, stop=False,
                    )
            for o in range(4):
                psA, psB = ph1[o]
                lhsT = w_t[3][:, o * P : (o + 1) * P]
                nc.tensor.matmul(
                    psA[:], lhsT, d_t[3][0][:, 0:512],
                    start=False, stop=True,
                )
                nc.tensor.matmul(
                    psB[:], lhsT, d_t[3][0][:, 512:QCOL],
                    start=False, stop=True,
                )
                emit_evac(psA, psB, 0, o)

            # Phase 2 — everything else in normal order (bf16 k-major, DR
            # tail) since all operands are SBUF-resident by then.
            for q in range(NQ):
                for o in range(4 if q == 0 else 0, NO):
                    psA = pp.tile([P, 512], mybir.dt.float32, tag="pa", name="psA")
                    psB = pp.tile([P, 512], mybir.dt.float32, tag="pb", name="psB")
                    xdeep = q == 2 and o >= 4
                    for k in range(4 if xdeep else kmax(q)):
                        lhsT = w_t[k][:, o * P : (o + 1) * P]
                        nc.tensor.matmul(
                            psA[:], lhsT, d_t[k][q][:, 0:512],
                            start=(k == 0), stop=False,
                        )
                        nc.tensor.matmul(
                            psB[:], lhsT, d_t[k][q][:, 512:QCOL],
                            start=(k == 0), stop=False,
                        )
                    emit_dr(psA, psB, q, o, first=False, xdeep=xdeep)
                    emit_evac(psA, psB, q, o)

    nc.compile()
    return nc


def _get_nc():
    if "nc" not in _CACHE:
        _CACHE["nc"] = _build()
    return _CACHE["nc"]


def _prep_weights(W, b):
    W = np.asarray(W, dtype=np.float32)
    b = np.asarray(b, dtype=np.float32)
    Ws = W * SCALE
    # wT[k, p, o] = W[o, k*128+p] * 128  (bf16)
    wT = np.ascontiguousarray(
        Ws[:, : KB * P].T.reshape(KB, P, OUT_DIM).astype(BF)
    )
    # w8lo[p, i, o] = e4m3(W[o, 512 + i*128 + p] * 128)  (k-blocks 4,5)
    # w8hi[p, i, o] = e4m3(W[o, 768 + i*128 + p] * 128)  (k-blocks 6,7)
    w8lo = np.ascontiguousarray(
        Ws[:, 4 * P : 6 * P].T.reshape(2, P, OUT_DIM).transpose(1, 0, 2).astype(E4)
    )
    w8hi = np.ascontiguousarray(
        Ws[:, 6 * P :].T.reshape(2, P, OUT_DIM).transpose(1, 0, 2).astype(E4)
    )
    bias2 = np.ascontiguousarray(b.reshape(NO, P).T)  # [128, 8] f32
    return wT, w8lo, w8hi, bias2


def _prep_inputs(data, W, b):
    data = np.asarray(data, dtype=np.float32)
    wT, w8lo, w8hi, bias2 = _prep_weights(W, b)
    in_maps = []
    for c in range(N_CORES):
        shard = data[c * SHARD : (c + 1) * SHARD]  # [4096, 1024] f32
        # dT[k, p, b] = bf16(shard[b, k*128+p])
        dTc = np.ascontiguousarray(
            shard[:, : KB * P].T.reshape(KB, P, SHARD).astype(BF)
        )
        # d8a: shallow quarters q1, q2 (rows 1024:3072), k-blocks 6,7
        d8at = shard[QCOL : 3 * QCOL, 6 * P :].T.reshape(2, P, 2, QCOL)
        d8ac = np.ascontiguousarray(d8at.transpose(2, 1, 0, 3).astype(E4))
        # d8b: deep quarters q0, q3 (rows 0:1024 and 3072:4096), k-blocks
        # 4..7
        deep_rows = np.concatenate(
            [shard[:QCOL, 4 * P :], shard[3 * QCOL :, 4 * P :]]
        )
        d8bt = deep_rows.T.reshape(4, P, 2, QCOL)
        d8bc = np.ascontiguousarray(d8bt.transpose(2, 1, 0, 3).astype(E4))
        # d8c: q2 rows (2048:3072), k-blocks 4,5 -- extra deep groups o>=6
        d8ct = shard[2 * QCOL : 3 * QCOL, 4 * P : 6 * P].T.reshape(2, P, QCOL)
        d8cc = np.ascontiguousarray(d8ct.transpose(1, 0, 2).astype(E4))
        in_maps.append(
            {"dT": dTc, "d8a": d8ac, "d8b": d8bc, "d8c": d8cc, "wT": wT,
             "w8lo": w8lo, "w8hi": w8hi, "biasb": bias2}
        )
    return in_maps


def _run(data, W, b, trace=False, **trace_kw):
    nc = _get_nc()
    in_maps = _prep_inputs(data, W, b)
    res = run_bass_kernel_spmd(
        nc, in_maps, list(range(N_CORES)), trace=trace, **trace_kw
    )
    out = np.concatenate(
        [
            np.asarray(res.results[c]["outT"]).T.astype(np.float32)
            for c in range(N_CORES)
        ],
        axis=0,
    )
    return out, res


def kernel(**inputs) -> np.ndarray:
    out, _ = _run(inputs["data"], inputs["W"], inputs["b"])
    return out


# revision 28
# speedup vs baseline: 1.0390x; 1.0040x over previous
"""GroupFC kernel for Trainium2, data-parallel across 8 NeuronCores.

Problem: out = data @ W.T + b
  data: [32768, 1024] f32, W: [1024, 1024] f32, b: [1024] f32

Strategy:
  - Shard batch dim across 8 cores (4096 rows each); replicate W, b.
  - Transposed-output formulation: outT[o, b] = sum_k W[o,k] d[b,k] + b[o].
    Stationary operand = W tiles (out-dim on PSUM partitions), moving
    operand = data columns (batch on the free dim).
  - Mixed precision along the contraction, tuned to the 2e-2 rel-err
    budget: two batch quarters (q1, q2) run k-blocks 0..5 in bf16
    (1 col/cycle) and blocks 6,7 in fp8-e4m3 DoubleRow (measured ~2x
    column rate); the other two (q0, q3) run blocks 0..3 bf16 and 4..7
    fp8 (two DR pairs). Measured rel err ~1.87e-2.
  - q0 is a deep-fp8 quarter on purpose: the DR-first ramp phase gets
    ~2x the fp8 work from ~1 MiB of loads, so the PE is busy while the
    bf16 tiles stream in, and the bf16 k-outer ramp is 4 steps, not 6.
  - All W values pre-scaled by 128 on the host so the fp8 weights avoid
    the e4m3 subnormal range; the fused evacuation applies 1/128 and the
    per-out-row bias in one pass per bank (DVE; ACT joins for the last
    quarter), emitting bf16 halves stored immediately on both HWDGE
    rings.
  - Startup: memset-gated warmup matmuls ramp the PE HAM clock gate.
  - Host post-pass transposes outT back to [batch, out] f32.
"""

import sys
from contextlib import ExitStack

import numpy as np

try:
    import concourse.bass as bass  # noqa: F401
except ImportError:
    sys.path.insert(0, "/opt/trn_rl_repo")

import ml_dtypes

import concourse.tile as tile
from concourse import bacc, mybir
from concourse.bass_utils import run_bass_kernel_spmd

N_CORES = 8
BATCH = 32768
SHARD = BATCH // N_CORES  # 4096
IN_DIM = 1024
OUT_DIM = 1024
P = 128
KB = 6  # bf16 k-blocks for shallow quarters; deep quarters use KB-2
NQ = 4  # batch quarters per core (1024 columns each)
QCOL = SHARD // NQ  # 1024
NO = OUT_DIM // P  # 8 output-row blocks
SCALE = 128.0
DEEP = (True, False, False, True)  # per-quarter: 4 fp8 k-blocks vs 2
E4 = ml_dtypes.float8_e4m3
BF = ml_dtypes.bfloat16

_CACHE = {}


def _build():
    nc = bacc.Bacc("TRN2", target_bir_lowering=False, debug=False)
    dT = nc.dram_tensor(
        "dT", [KB, P, SHARD], mybir.dt.bfloat16, kind="ExternalInput"
    ).ap()
    d8a = nc.dram_tensor(  # shallow quarters (q1, q2), k-blocks 6,7
        "d8a", [2, P, 2, QCOL], mybir.dt.float8e4, kind="ExternalInput"
    ).ap()
    d8b = nc.dram_tensor(  # deep quarters (q0, q3), k-blocks 4..7
        "d8b", [2, P, 4, QCOL], mybir.dt.float8e4, kind="ExternalInput"
    ).ap()
    d8c = nc.dram_tensor(  # extra deep groups (q2, o>=6), k-blocks 4,5
        "d8c", [P, 2, QCOL], mybir.dt.float8e4, kind="ExternalInput"
    ).ap()
    wT = nc.dram_tensor(
        "wT", [KB, P, OUT_DIM], mybir.dt.bfloat16, kind="ExternalInput"
    ).ap()
    w8hi = nc.dram_tensor(  # fp8 weights, k-blocks 6,7
        "w8hi", [P, 2, OUT_DIM], mybir.dt.float8e4, kind="ExternalInput"
    ).ap()
    w8lo = nc.dram_tensor(  # fp8 weights, k-blocks 4,5
        "w8lo", [P, 2, OUT_DIM], mybir.dt.float8e4, kind="ExternalInput"
    ).ap()
    biasb = nc.dram_tensor(
        "biasb", [P, NO], mybir.dt.float32, kind="ExternalInput"
    ).ap()
    outT = nc.dram_tensor(
        "outT", [OUT_DIM, SHARD], mybir.dt.bfloat16, kind="ExternalOutput"
    ).ap()

    with tile.TileContext(nc) as tc:
        with ExitStack() as ctx:
            wp = ctx.enter_context(tc.tile_pool(name="w", bufs=1))
            dp = ctx.enter_context(tc.tile_pool(name="d", bufs=1))
            bp = ctx.enter_context(tc.tile_pool(name="misc", bufs=1))
            pp = ctx.enter_context(tc.tile_pool(name="psum", bufs=4, space="PSUM"))
            op = ctx.enter_context(tc.tile_pool(name="o", bufs=8))

            w_t = [None] * KB
            d_t = [[None] * NQ for _ in range(KB)]
            d8_t = [None] * NQ  # q0 uses the half tiles below instead
            tiles = {}

            def kmax(q):
                return KB - 2 if DEEP[q] else KB

            # Load plan. Critical ramp first: the fp8 weights (both pairs)
            # and the q0 fp8 data halves unlock the DR-first phase from
            # ~1.1 MiB; then (wT[k], dT[k] q0) pairs k=0..3 in consumption
            # order; then the rest. Alternate the two HWDGE rings.
            loads = [
                ("bias", 0, 0), ("w8hiA", 0, 0), ("d8q0A", 0, 0),
                ("w8lo", 0, 0), ("d8q0B", 0, 0), ("w8hiB", 0, 0),
            ]
            for k in range(4):
                loads.append(("w", k, 0))
                loads.append(("d", k, 0))
            loads += [("w", 4, 0), ("w", 5, 0)]
            for q in range(1, NQ):
                for k in range(kmax(q)):
                    loads.append(("d", k, q))
                loads.append(("d8", 0, q))
                if q == 2:
                    loads.append(("d8c", 0, 0))

            for i, (kind, k, q) in enumerate(loads):
                eng = nc.scalar if i % 2 == 0 else nc.sync
                if kind == "w":
                    w_t[k] = wp.tile([P, OUT_DIM], mybir.dt.bfloat16, tag=f"w{k}", name=f"w_t{k}")
                    eng.dma_start(out=w_t[k][:], in_=wT[k, :, :])
                elif kind == "d":
                    d_t[k][q] = dp.tile([P, QCOL], mybir.dt.bfloat16, tag=f"d{k}_{q}", name=f"d_t{k}_{q}")
                    eng.dma_start(
                        out=d_t[k][q][:], in_=dT[k, :, q * QCOL : (q + 1) * QCOL]
                    )
                elif kind == "w8hiA":
                    tiles["w8hiA"] = wp.tile([P, 2, 512], mybir.dt.float8e4, tag="w8hiA", name="w8hiA_t")
                    eng.dma_start(out=tiles["w8hiA"][:], in_=w8hi[:, :, 0:512])
                elif kind == "w8hiB":
                    tiles["w8hiB"] = wp.tile([P, 2, 512], mybir.dt.float8e4, tag="w8hiB", name="w8hiB_t")
                    eng.dma_start(out=tiles["w8hiB"][:], in_=w8hi[:, :, 512:OUT_DIM])
                elif kind == "w8lo":
                    tiles["w8lo"] = wp.tile([P, 2, OUT_DIM], mybir.dt.float8e4, tag="w8lo", name="w8lo_t")
                    eng.dma_start(out=tiles["w8lo"][:], in_=w8lo[:, :, :])
                elif kind == "d8q0A":
                    tiles["d8q0A"] = dp.tile([P, 4, 512], mybir.dt.float8e4, tag="d8q0A", name="d8q0A_t")
                    eng.dma_start(out=tiles["d8q0A"][:], in_=d8b[0, :, :, 0:512])
                elif kind == "d8q0B":
                    tiles["d8q0B"] = dp.tile([P, 4, 512], mybir.dt.float8e4, tag="d8q0B", name="d8q0B_t")
                    eng.dma_start(out=tiles["d8q0B"][:], in_=d8b[0, :, :, 512:QCOL])
                elif kind == "d8c":
                    tiles["d8c"] = dp.tile([P, 2, QCOL], mybir.dt.float8e4, tag="d8c", name="d8c_t")
                    eng.dma_start(out=tiles["d8c"][:], in_=d8c[:, :, :])
                elif kind == "d8":
                    nblk = 4 if DEEP[q] else 2
                    d8_t[q] = dp.tile([P, nblk, QCOL], mybir.dt.float8e4, tag=f"d8_{q}", name=f"d8_t{q}")
                    src = d8b[1] if DEEP[q] else d8a[q - 1]
                    eng.dma_start(out=d8_t[q][:], in_=src[:, :, :])
                else:
                    bias_t = bp.tile([P, NO], mybir.dt.float32, tag="bias", name="bias_t")
                    eng.dma_start(out=bias_t[:], in_=biasb[:, :])

            # Warmup: ramp the PE HAM clock while loads stream. Gated on an
            # on-chip memset so it starts as soon as the engines come up.
            scr = bp.tile([P, 256], mybir.dt.bfloat16, tag="scr", name="scr")
            nc.vector.memset(scr[:], 0)
            ps_first = [
                pp.tile([P, 512], mybir.dt.float32, tag="pa", name="ps_a0"),
                pp.tile([P, 512], mybir.dt.float32, tag="pb", name="ps_b0"),
            ]
            for i in range(24):
                nc.tensor.matmul(
                    ps_first[0][:, 0:256], scr[:, 0:P], scr[:],
                    start=True, stop=True, skip_group_check=True,
                )

            dr = mybir.MatmulPerfMode.DoubleRow

            def whi(o):
                t = tiles["w8hiA"] if o < 4 else tiles["w8hiB"]
                return t[:, :, (o % 4) * P : (o % 4 + 1) * P]

            def emit_dr(psA, psB, q, o, first, xdeep=False):
                # Only the FIRST matmul per bank may set start=True: start
                # clears has_written for the WHOLE bank, so a second start on
                # the other half would wipe the first half's result.
                osl = slice(o * P, (o + 1) * P)
                npair = 2 if (DEEP[q] or xdeep) else 1
                for gi in range(npair):
                    if DEEP[q]:
                        wsl = tiles["w8lo"][:, :, osl] if gi == 0 else whi(o)
                        dlo = 2 * gi
                    elif xdeep:
                        wsl = tiles["w8lo"][:, :, osl] if gi == 0 else whi(o)
                        dlo = 0
                    else:
                        wsl = whi(o)
                        dlo = 0
                    if xdeep and gi == 0:
                        dsl = tiles["d8c"]
                        srcs = [
                            dsl[:, 0:2, 0:256],
                            dsl[:, 0:2, 256:512],
                            dsl[:, 0:2, 512:768],
                            dsl[:, 0:2, 768:QCOL],
                        ]
                    elif q == 0:
                        srcs = [
                            tiles["d8q0A"][:, dlo : dlo + 2, 0:256],
                            tiles["d8q0A"][:, dlo : dlo + 2, 256:512],
                            tiles["d8q0B"][:, dlo : dlo + 2, 0:256],
                            tiles["d8q0B"][:, dlo : dlo + 2, 256:512],
                        ]
                    else:
                        dsl = d8_t[q]
                        srcs = [
                            dsl[:, dlo : dlo + 2, 0:256],
                            dsl[:, dlo : dlo + 2, 256:512],
                            dsl[:, dlo : dlo + 2, 512:768],
                            dsl[:, dlo : dlo + 2, 768:QCOL],
                        ]
                    last = gi == npair - 1
                    st = first and gi == 0
                    nc.tensor.matmul(
                        psA[:, 0:256], wsl, srcs[0],
                        start=st, stop=(not first) and last, perf_mode=dr,
                        skip_group_check=True,
                    )
                    nc.tensor.matmul(
                        psA[:, 256:512], wsl, srcs[1],
                        start=False, stop=(not first) and last, perf_mode=dr,
                        skip_group_check=True,
                    )
                    nc.tensor.matmul(
                        psB[:, 0:256], wsl, srcs[2],
                        start=st, stop=(not first) and last, perf_mode=dr,
                        skip_group_check=True,
                    )
                    nc.tensor.matmul(
                        psB[:, 256:512], wsl, srcs[3],
                        start=False, stop=(not first) and last, perf_mode=dr,
                        skip_group_check=True,
                    )

            def emit_evac(psA, psB, q, o):
                # Fused evacuation: out = psum/128 + bias[o], to bf16, with
                # separate half-tiles so each store only waits on its own
                # evacuation and rides its own HW ring. Evacs stay on DVE
                # (the ring queues carry the load DMAs); the last quarter
                # also uses ACT, whose queue has drained by then.
                osl = slice(o * P, (o + 1) * P)
                bcol = bias_t[:, o : o + 1]
                c0 = q * QCOL
                osbA = op.tile([P, 512], mybir.dt.bfloat16, tag="osbA", name="osbA")
                if q == NQ - 1:
                    nc.scalar.activation(
                        osbA[:], psA[:],
                        mybir.ActivationFunctionType.Identity,
                        bias=bcol, scale=1.0 / SCALE,
                    )
                else:
                    nc.vector.tensor_scalar(
                        osbA[:], psA[:],
                        1.0 / SCALE, bcol,
                        mybir.AluOpType.mult, mybir.AluOpType.add,
                    )
                nc.scalar.dma_start(out=outT[osl, c0 : c0 + 512], in_=osbA[:])
                osbB = op.tile([P, 512], mybir.dt.bfloat16, tag="osbB", name="osbB")
                nc.vector.tensor_scalar(
                    osbB[:], psB[:],
                    1.0 / SCALE, bcol,
                    mybir.AluOpType.mult, mybir.AluOpType.add,
                )
                nc.sync.dma_start(out=outT[osl, c0 + 512 : c0 + QCOL], in_=osbB[:])

            # Phase 1 — groups (q0, o=0..3), DR-first. q0 is deep-fp8: each
            # group opens with 8 DR matmuls (pairs 4,5 and 6,7). The psA
            # sides run first across the groups (they need only the A
            # halves of the fp8 data), then psB; then the bf16 part runs
            # k-outer, matching DMA arrival, with the final k-step staggered
            # per group for bubble-free PSUM recycling into phase 2.
            ph1 = []
            for o in range(4):
                psA, psB = ps_first if o == 0 else (
                    pp.tile([P, 512], mybir.dt.float32, tag="pa", name="psA"),
                    pp.tile([P, 512], mybir.dt.float32, tag="pb", name="psB"),
                )
                ph1.append((psA, psB))
            for half in range(2):
                dh = tiles["d8q0A"] if half == 0 else tiles["d8q0B"]
                for gi in range(2):
                    for o in range(4):
                        ps = ph1[o][half]
                        wsl = (tiles["w8lo"][:, :, o * P : (o + 1) * P]
                               if gi == 0 else whi(o))
                        nc.tensor.matmul(
                            ps[:, 0:256], wsl, dh[:, 2 * gi : 2 * gi + 2, 0:256],
                            start=(gi == 0), stop=False, perf_mode=dr,
                            skip_group_check=True,
                        )
                        nc.tensor.matmul(
                            ps[:, 256:512], wsl, dh[:, 2 * gi : 2 * gi + 2, 256:512],
                            start=False, stop=False, perf_mode=dr,
                            skip_group_check=True,
                        )
            for k in range(3):
                for o in range(4):
                    psA, psB = ph1[o]
                    lhsT = w_t[k][:, o * P : (o + 1) * P]
                    nc.tensor.matmul(
                        psA[:], lhsT, d_t[k][0][:, 0:512],
                        start=False, stop=False,
                    )
                    nc.tensor.matmul(
                        psB[:], lhsT, d_t[k][0][:, 512:QCOL],
                        start=False, stop=False,
                    )
            for o in range(4):
                psA, psB = ph1[o]
                lhsT = w_t[3][:, o * P : (o + 1) * P]
                nc.tensor.matmul(
                    psA[:], lhsT, d_t[3][0][:, 0:512],
                    start=False, stop=True,
                )
                nc.tensor.matmul(
                    psB[:], lhsT, d_t[3][0][:, 512:QCOL],
                    start=False, stop=True,
                )
                emit_evac(psA, psB, 0, o)

            # Phase 2 — everything else in normal order (bf16 k-major, DR
            # tail) since all operands are SBUF-resident by then.
            for q in range(NQ):
                for o in range(4 if q == 0 else 0, NO):
                    psA = pp.tile([P, 512], mybir.dt.float32, tag="pa", name="psA")
                    psB = pp.tile([P, 512], mybir.dt.float32, tag="pb", name="psB")
                    xdeep = q == 2 and o >= 3
                    for k in range(4 if xdeep else kmax(q)):
                        lhsT = w_t[k][:, o * P : (o + 1) * P]
                        nc.tensor.matmul(
                            psA[:], lhsT, d_t[k][q][:, 0:512],
                            start=(k == 0), stop=False,
                        )
                        nc.tensor.matmul(
                            psB[:], lhsT, d_t[k][q][:, 512:QCOL],
                            start=(k == 0), stop=False,
                        )
                    emit_dr(psA, psB, q, o, first=False, xdeep=xdeep)
                    emit_evac(psA, psB, q, o)

    nc.compile()
    return nc


def _get_nc():
    if "nc" not in _CACHE:
        _CACHE["nc"] = _build()
    return _CACHE["nc"]


def _prep_weights(W, b):
    W = np.asarray(W, dtype=np.float32)
    b = np.asarray(b, dtype=np.float32)
    Ws = W * SCALE
    # wT[k, p, o] = W[o, k*128+p] * 128  (bf16)
    wT = np.ascontiguousarray(
        Ws[:, : KB * P].T.reshape(KB, P, OUT_DIM).astype(BF)
    )
    # w8lo[p, i, o] = e4m3(W[o, 512 + i*128 + p] * 128)  (k-blocks 4,5)
    # w8hi[p, i, o] = e4m3(W[o, 768 + i*128 + p] * 128)  (k-blocks 6,7)
    w8lo = np.ascontiguousarray(
        Ws[:, 4 * P : 6 * P].T.reshape(2, P, OUT_DIM).transpose(1, 0, 2).astype(E4)
    )
    w8hi = np.ascontiguousarray(
        Ws[:, 6 * P :].T.reshape(2, P, OUT_DIM).transpose(1, 0, 2).astype(E4)
    )
    bias2 = np.ascontiguousarray(b.reshape(NO, P).T)  # [128, 8] f32
    return wT, w8lo, w8hi, bias2


def _prep_inputs(data, W, b):
    data = np.asarray(data, dtype=np.float32)
    wT, w8lo, w8hi, bias2 = _prep_weights(W, b)
    in_maps = []
    for c in range(N_CORES):
        shard = data[c * SHARD : (c + 1) * SHARD]  # [4096, 1024] f32
        # dT[k, p, b] = bf16(shard[b, k*128+p])
        dTc = np.ascontiguousarray(
            shard[:, : KB * P].T.reshape(KB, P, SHARD).astype(BF)
        )
        # d8a: shallow quarters q1, q2 (rows 1024:3072), k-blocks 6,7
        d8at = shard[QCOL : 3 * QCOL, 6 * P :].T.reshape(2, P, 2, QCOL)
        d8ac = np.ascontiguousarray(d8at.transpose(2, 1, 0, 3).astype(E4))
        # d8b: deep quarters q0, q3 (rows 0:1024 and 3072:4096), k-blocks
        # 4..7
        deep_rows = np.concatenate(
            [shard[:QCOL, 4 * P :], shard[3 * QCOL :, 4 * P :]]
        )
        d8bt = deep_rows.T.reshape(4, P, 2, QCOL)
        d8bc = np.ascontiguousarray(d8bt.transpose(2, 1, 0, 3).astype(E4))
        # d8c: q2 rows (2048:3072), k-blocks 4,5 -- extra deep groups o>=6
        d8ct = shard[2 * QCOL : 3 * QCOL, 4 * P : 6 * P].T.reshape(2, P, QCOL)
        d8cc = np.ascontiguousarray(d8ct.transpose(1, 0, 2).astype(E4))
        in_maps.append(
            {"dT": dTc, "d8a": d8ac, "d8b": d8bc, "d8c": d8cc, "wT": wT,
             "w8lo": w8lo, "w8hi": w8hi, "biasb": bias2}
        )
    return in_maps


def _run(data, W, b, trace=False, **trace_kw):
    nc = _get_nc()
    in_maps = _prep_inputs(data, W, b)
    res = run_bass_kernel_spmd(
        nc, in_maps, list(range(N_CORES)), trace=trace, **trace_kw
    )
    out = np.concatenate(
        [
            np.asarray(res.results[c]["outT"]).T.astype(np.float32)
            for c in range(N_CORES)
        ],
        axis=0,
    )
    return out, res


def kernel(**inputs) -> np.ndarray:
    out, _ = _run(inputs["data"], inputs["W"], inputs["b"])
    return out


# revision 29
# speedup vs baseline: 1.0403x; 1.0012x over previous
"""GroupFC kernel for Trainium2, data-parallel across 8 NeuronCores.

Problem: out = data @ W.T + b
  data: [32768, 1024] f32, W: [1024, 1024] f32, b: [1024] f32

Strategy:
  - Shard batch dim across 8 cores (4096 rows each); replicate W, b.
  - Transposed-output formulation: outT[o, b] = sum_k W[o,k] d[b,k] + b[o].
    Stationary operand = W tiles (out-dim on PSUM partitions), moving
    operand = data columns (batch on the free dim).
  - Mixed precision along the contraction, tuned to the 2e-2 rel-err
    budget: two batch quarters (q1, q2) run k-blocks 0..5 in bf16
    (1 col/cycle) and blocks 6,7 in fp8-e4m3 DoubleRow (measured ~2x
    column rate); the other two (q0, q3) run blocks 0..3 bf16 and 4..7
    fp8 (two DR pairs). Measured rel err ~1.87e-2.
  - q0 is a deep-fp8 quarter on purpose: the DR-first ramp phase gets
    ~2x the fp8 work from ~1 MiB of loads, so the PE is busy while the
    bf16 tiles stream in, and the bf16 k-outer ramp is 4 steps, not 6.
  - All W values pre-scaled by 128 on the host so the fp8 weights avoid
    the e4m3 subnormal range; the fused evacuation applies 1/128 and the
    per-out-row bias in one pass per bank (DVE; ACT joins for the last
    quarter), emitting bf16 halves stored immediately on both HWDGE
    rings.
  - Startup: memset-gated warmup matmuls ramp the PE HAM clock gate.
  - Host post-pass transposes outT back to [batch, out] f32.
"""

import sys
from contextlib import ExitStack

import numpy as np

try:
    import concourse.bass as bass  # noqa: F401
except ImportError:
    sys.path.insert(0, "/opt/trn_rl_repo")

import ml_dtypes

import concourse.tile as tile
from concourse import bacc, mybir
from concourse.bass_utils import run_bass_kernel_spmd

N_CORES = 8
BATCH = 32768
SHARD = BATCH // N_CORES  # 4096
IN_DIM = 1024
OUT_DIM = 1024
P = 128
KB = 6  # bf16 k-blocks for shallow quarters; deep quarters use KB-2
NQ = 4  # batch quarters per core (1024 columns each)
QCOL = SHARD // NQ  # 1024
NO = OUT_DIM // P  # 8 output-row blocks
SCALE = 128.0
DEEP = (True, False, False, True)  # per-quarter: 4 fp8 k-blocks vs 2
E4 = ml_dtypes.float8_e4m3
BF = ml_dtypes.bfloat16

_CACHE = {}


def _build():
    nc = bacc.Bacc("TRN2", target_bir_lowering=False, debug=False)
    dT = nc.dram_tensor(
        "dT", [KB, P, SHARD], mybir.dt.bfloat16, kind="ExternalInput"
    ).ap()
    d8a = nc.dram_tensor(  # shallow quarters (q1, q2), k-blocks 6,7
        "d8a", [2, P, 2, QCOL], mybir.dt.float8e4, kind="ExternalInput"
    ).ap()
    d8b = nc.dram_tensor(  # deep quarters (q0, q3), k-blocks 4..7
        "d8b", [2, P, 4, QCOL], mybir.dt.float8e4, kind="ExternalInput"
    ).ap()
    d8c = nc.dram_tensor(  # extra deep groups (q2, o>=6), k-blocks 4,5
        "d8c", [P, 2, QCOL], mybir.dt.float8e4, kind="ExternalInput"
    ).ap()
    wT = nc.dram_tensor(
        "wT", [KB, P, OUT_DIM], mybir.dt.bfloat16, kind="ExternalInput"
    ).ap()
    w8hi = nc.dram_tensor(  # fp8 weights, k-blocks 6,7
        "w8hi", [P, 2, OUT_DIM], mybir.dt.float8e4, kind="ExternalInput"
    ).ap()
    w8lo = nc.dram_tensor(  # fp8 weights, k-blocks 4,5
        "w8lo", [P, 2, OUT_DIM], mybir.dt.float8e4, kind="ExternalInput"
    ).ap()
    biasb = nc.dram_tensor(
        "biasb", [P, NO], mybir.dt.float32, kind="ExternalInput"
    ).ap()
    outT = nc.dram_tensor(
        "outT", [OUT_DIM, SHARD], mybir.dt.bfloat16, kind="ExternalOutput"
    ).ap()

    with tile.TileContext(nc) as tc:
        with ExitStack() as ctx:
            wp = ctx.enter_context(tc.tile_pool(name="w", bufs=1))
            dp = ctx.enter_context(tc.tile_pool(name="d", bufs=1))
            bp = ctx.enter_context(tc.tile_pool(name="misc", bufs=1))
            pp = ctx.enter_context(tc.tile_pool(name="psum", bufs=4, space="PSUM"))
            op = ctx.enter_context(tc.tile_pool(name="o", bufs=8))

            w_t = [None] * KB
            d_t = [[None] * NQ for _ in range(KB)]
            d8_t = [None] * NQ  # q0 uses the half tiles below instead
            tiles = {}

            def kmax(q):
                return KB - 2 if DEEP[q] else KB

            # Load plan. Critical ramp first: the fp8 weights (both pairs)
            # and the q0 fp8 data halves unlock the DR-first phase from
            # ~1.1 MiB; then (wT[k], dT[k] q0) pairs k=0..3 in consumption
            # order; then the rest. Alternate the two HWDGE rings.
            loads = [
                ("bias", 0, 0), ("w8hiA", 0, 0), ("d8q0A", 0, 0),
                ("w8lo", 0, 0), ("d8q0B", 0, 0), ("w8hiB", 0, 0),
            ]
            for k in range(4):
                loads.append(("w", k, 0))
                loads.append(("d", k, 0))
            loads += [("w", 4, 0), ("w", 5, 0)]
            for q in range(1, NQ):
                for k in range(kmax(q)):
                    loads.append(("d", k, q))
                loads.append(("d8", 0, q))
                if q == 2:
                    loads.append(("d8c", 0, 0))

            for i, (kind, k, q) in enumerate(loads):
                eng = nc.scalar if i % 2 == 0 else nc.sync
                if kind == "w":
                    w_t[k] = wp.tile([P, OUT_DIM], mybir.dt.bfloat16, tag=f"w{k}", name=f"w_t{k}")
                    eng.dma_start(out=w_t[k][:], in_=wT[k, :, :])
                elif kind == "d":
                    d_t[k][q] = dp.tile([P, QCOL], mybir.dt.bfloat16, tag=f"d{k}_{q}", name=f"d_t{k}_{q}")
                    eng.dma_start(
                        out=d_t[k][q][:], in_=dT[k, :, q * QCOL : (q + 1) * QCOL]
                    )
                elif kind == "w8hiA":
                    tiles["w8hiA"] = wp.tile([P, 2, 512], mybir.dt.float8e4, tag="w8hiA", name="w8hiA_t")
                    eng.dma_start(out=tiles["w8hiA"][:], in_=w8hi[:, :, 0:512])
                elif kind == "w8hiB":
                    tiles["w8hiB"] = wp.tile([P, 2, 512], mybir.dt.float8e4, tag="w8hiB", name="w8hiB_t")
                    eng.dma_start(out=tiles["w8hiB"][:], in_=w8hi[:, :, 512:OUT_DIM])
                elif kind == "w8lo":
                    tiles["w8lo"] = wp.tile([P, 2, OUT_DIM], mybir.dt.float8e4, tag="w8lo", name="w8lo_t")
                    eng.dma_start(out=tiles["w8lo"][:], in_=w8lo[:, :, :])
                elif kind == "d8q0A":
                    tiles["d8q0A"] = dp.tile([P, 4, 512], mybir.dt.float8e4, tag="d8q0A", name="d8q0A_t")
                    eng.dma_start(out=tiles["d8q0A"][:], in_=d8b[0, :, :, 0:512])
                elif kind == "d8q0B":
                    tiles["d8q0B"] = dp.tile([P, 4, 512], mybir.dt.float8e4, tag="d8q0B", name="d8q0B_t")
                    eng.dma_start(out=tiles["d8q0B"][:], in_=d8b[0, :, :, 512:QCOL])
                elif kind == "d8c":
                    tiles["d8c"] = dp.tile([P, 2, QCOL], mybir.dt.float8e4, tag="d8c", name="d8c_t")
                    eng.dma_start(out=tiles["d8c"][:], in_=d8c[:, :, :])
                elif kind == "d8":
                    nblk = 4 if DEEP[q] else 2
                    d8_t[q] = dp.tile([P, nblk, QCOL], mybir.dt.float8e4, tag=f"d8_{q}", name=f"d8_t{q}")
                    src = d8b[1] if DEEP[q] else d8a[q - 1]
                    eng.dma_start(out=d8_t[q][:], in_=src[:, :, :])
                else:
                    bias_t = bp.tile([P, NO], mybir.dt.float32, tag="bias", name="bias_t")
                    eng.dma_start(out=bias_t[:], in_=biasb[:, :])

            # Warmup: ramp the PE HAM clock while loads stream. Gated on an
            # on-chip memset so it starts as soon as the engines come up.
            scr = bp.tile([P, 256], mybir.dt.bfloat16, tag="scr", name="scr")
            nc.vector.memset(scr[:], 0)
            ps_first = [
                pp.tile([P, 512], mybir.dt.float32, tag="pa", name="ps_a0"),
                pp.tile([P, 512], mybir.dt.float32, tag="pb", name="ps_b0"),
            ]
            for i in range(20):
                nc.tensor.matmul(
                    ps_first[0][:, 0:256], scr[:, 0:P], scr[:],
                    start=True, stop=True, skip_group_check=True,
                )

            dr = mybir.MatmulPerfMode.DoubleRow

            def whi(o):
                t = tiles["w8hiA"] if o < 4 else tiles["w8hiB"]
                return t[:, :, (o % 4) * P : (o % 4 + 1) * P]

            def emit_dr(psA, psB, q, o, first, xdeep=False):
                # Only the FIRST matmul per bank may set start=True: start
                # clears has_written for the WHOLE bank, so a second start on
                # the other half would wipe the first half's result.
                osl = slice(o * P, (o + 1) * P)
                npair = 2 if (DEEP[q] or xdeep) else 1
                for gi in range(npair):
                    if DEEP[q]:
                        wsl = tiles["w8lo"][:, :, osl] if gi == 0 else whi(o)
                        dlo = 2 * gi
                    elif xdeep:
                        wsl = tiles["w8lo"][:, :, osl] if gi == 0 else whi(o)
                        dlo = 0
                    else:
                        wsl = whi(o)
                        dlo = 0
                    if xdeep and gi == 0:
                        dsl = tiles["d8c"]
                        srcs = [
                            dsl[:, 0:2, 0:256],
                            dsl[:, 0:2, 256:512],
                            dsl[:, 0:2, 512:768],
                            dsl[:, 0:2, 768:QCOL],
                        ]
                    elif q == 0:
                        srcs = [
                            tiles["d8q0A"][:, dlo : dlo + 2, 0:256],
                            tiles["d8q0A"][:, dlo : dlo + 2, 256:512],
                            tiles["d8q0B"][:, dlo : dlo + 2, 0:256],
                            tiles["d8q0B"][:, dlo : dlo + 2, 256:512],
                        ]
                    else:
                        dsl = d8_t[q]
                        srcs = [
                            dsl[:, dlo : dlo + 2, 0:256],
                            dsl[:, dlo : dlo + 2, 256:512],
                            dsl[:, dlo : dlo + 2, 512:768],
                            dsl[:, dlo : dlo + 2, 768:QCOL],
                        ]
                    last = gi == npair - 1
                    st = first and gi == 0
                    nc.tensor.matmul(
                        psA[:, 0:256], wsl, srcs[0],
                        start=st, stop=(not first) and last, perf_mode=dr,
                        skip_group_check=True,
                    )
                    nc.tensor.matmul(
                        psA[:, 256:512], wsl, srcs[1],
                        start=False, stop=(not first) and last, perf_mode=dr,
                        skip_group_check=True,
                    )
                    nc.tensor.matmul(
                        psB[:, 0:256], wsl, srcs[2],
                        start=st, stop=(not first) and last, perf_mode=dr,
                        skip_group_check=True,
                    )
                    nc.tensor.matmul(
                        psB[:, 256:512], wsl, srcs[3],
                        start=False, stop=(not first) and last, perf_mode=dr,
                        skip_group_check=True,
                    )

            def emit_evac(psA, psB, q, o):
                # Fused evacuation: out = psum/128 + bias[o], to bf16, with
                # separate half-tiles so each store only waits on its own
                # evacuation and rides its own HW ring. Evacs stay on DVE
                # (the ring queues carry the load DMAs); the last quarter
                # also uses ACT, whose queue has drained by then.
                osl = slice(o * P, (o + 1) * P)
                bcol = bias_t[:, o : o + 1]
                c0 = q * QCOL
                osbA = op.tile([P, 512], mybir.dt.bfloat16, tag="osbA", name="osbA")
                if q == NQ - 1:
                    nc.scalar.activation(
                        osbA[:], psA[:],
                        mybir.ActivationFunctionType.Identity,
                        bias=bcol, scale=1.0 / SCALE,
                    )
                else:
                    nc.vector.tensor_scalar(
                        osbA[:], psA[:],
                        1.0 / SCALE, bcol,
                        mybir.AluOpType.mult, mybir.AluOpType.add,
                    )
                nc.scalar.dma_start(out=outT[osl, c0 : c0 + 512], in_=osbA[:])
                osbB = op.tile([P, 512], mybir.dt.bfloat16, tag="osbB", name="osbB")
                nc.vector.tensor_scalar(
                    osbB[:], psB[:],
                    1.0 / SCALE, bcol,
                    mybir.AluOpType.mult, mybir.AluOpType.add,
                )
                nc.sync.dma_start(out=outT[osl, c0 + 512 : c0 + QCOL], in_=osbB[:])

            # Phase 1 — groups (q0, o=0..3), DR-first. q0 is deep-fp8: each
            # group opens with 8 DR matmuls (pairs 4,5 and 6,7). The psA
            # sides run first across the groups (they need only the A
            # halves of the fp8 data), then psB; then the bf16 part runs
            # k-outer, matching DMA arrival, with the final k-step staggered
            # per group for bubble-free PSUM recycling into phase 2.
            ph1 = []
            for o in range(4):
                psA, psB = ps_first if o == 0 else (
                    pp.tile([P, 512], mybir.dt.float32, tag="pa", name="psA"),
                    pp.tile([P, 512], mybir.dt.float32, tag="pb", name="psB"),
                )
                ph1.append((psA, psB))
            for half in range(2):
                dh = tiles["d8q0A"] if half == 0 else tiles["d8q0B"]
                for gi in range(2):
                    for o in range(4):
                        ps = ph1[o][half]
                        wsl = (tiles["w8lo"][:, :, o * P : (o + 1) * P]
                               if gi == 0 else whi(o))
                        nc.tensor.matmul(
                            ps[:, 0:256], wsl, dh[:, 2 * gi : 2 * gi + 2, 0:256],
                            start=(gi == 0), stop=False, perf_mode=dr,
                            skip_group_check=True,
                        )
                        nc.tensor.matmul(
                            ps[:, 256:512], wsl, dh[:, 2 * gi : 2 * gi + 2, 256:512],
                            start=False, stop=False, perf_mode=dr,
                            skip_group_check=True,
                        )
            for k in range(3):
                for o in range(4):
                    psA, psB = ph1[o]
                    lhsT = w_t[k][:, o * P : (o + 1) * P]
                    nc.tensor.matmul(
                        psA[:], lhsT, d_t[k][0][:, 0:512],
                        start=False, stop=False,
                    )
                    nc.tensor.matmul(
                        psB[:], lhsT, d_t[k][0][:, 512:QCOL],
                        start=False, stop=False,
                    )
            for o in range(4):
                psA, psB = ph1[o]
                lhsT = w_t[3][:, o * P : (o + 1) * P]
                nc.tensor.matmul(
                    psA[:], lhsT, d_t[3][0][:, 0:512],
                    start=False, stop=True,
                )
                nc.tensor.matmul(
                    psB[:], lhsT, d_t[3][0][:, 512:QCOL],
                    start=False, stop=True,
                )
                emit_evac(psA, psB, 0, o)

            # Phase 2 — everything else in normal order (bf16 k-major, DR
            # tail) since all operands are SBUF-resident by then.
            for q in range(NQ):
                for o in range(4 if q == 0 else 0, NO):
                    psA = pp.tile([P, 512], mybir.dt.float32, tag="pa", name="psA")
                    psB = pp.tile([P, 512], mybir.dt.float32, tag="pb", name="psB")
                    xdeep = q == 2 and o >= 3
                    for k in range(4 if xdeep else kmax(q)):
                        lhsT = w_t[k][:, o * P : (o + 1) * P]
                        nc.tensor.matmul(
                            psA[:], lhsT, d_t[k][q][:, 0:512],
                            start=(k == 0), stop=False,
                        )
                        nc.tensor.matmul(
                            psB[:], lhsT, d_t[k][q][:, 512:QCOL],
                            start=(k == 0), stop=False,
                        )
                    emit_dr(psA, psB, q, o, first=False, xdeep=xdeep)
                    emit_evac(psA, psB, q, o)

    nc.compile()
    return nc


def _get_nc():
    if "nc" not in _CACHE:
        _CACHE["nc"] = _build()
    return _CACHE["nc"]


def _prep_weights(W, b):
    W = np.asarray(W, dtype=np.float32)
    b = np.asarray(b, dtype=np.float32)
    Ws = W * SCALE
    # wT[k, p, o] = W[o, k*128+p] * 128  (bf16)
    wT = np.ascontiguousarray(
        Ws[:, : KB * P].T.reshape(KB, P, OUT_DIM).astype(BF)
    )
    # w8lo[p, i, o] = e4m3(W[o, 512 + i*128 + p] * 128)  (k-blocks 4,5)
    # w8hi[p, i, o] = e4m3(W[o, 768 + i*128 + p] * 128)  (k-blocks 6,7)
    w8lo = np.ascontiguousarray(
        Ws[:, 4 * P : 6 * P].T.reshape(2, P, OUT_DIM).transpose(1, 0, 2).astype(E4)
    )
    w8hi = np.ascontiguousarray(
        Ws[:, 6 * P :].T.reshape(2, P, OUT_DIM).transpose(1, 0, 2).astype(E4)
    )
    bias2 = np.ascontiguousarray(b.reshape(NO, P).T)  # [128, 8] f32
    return wT, w8lo, w8hi, bias2


def _prep_inputs(data, W, b):
    data = np.asarray(data, dtype=np.float32)
    wT, w8lo, w8hi, bias2 = _prep_weights(W, b)
    in_maps = []
    for c in range(N_CORES):
        shard = data[c * SHARD : (c + 1) * SHARD]  # [4096, 1024] f32
        # dT[k, p, b] = bf16(shard[b, k*128+p])
        dTc = np.ascontiguousarray(
            shard[:, : KB * P].T.reshape(KB, P, SHARD).astype(BF)
        )
        # d8a: shallow quarters q1, q2 (rows 1024:3072), k-blocks 6,7
        d8at = shard[QCOL : 3 * QCOL, 6 * P :].T.reshape(2, P, 2, QCOL)
        d8ac = np.ascontiguousarray(d8at.transpose(2, 1, 0, 3).astype(E4))
        # d8b: deep quarters q0, q3 (rows 0:1024 and 3072:4096), k-blocks
        # 4..7
        deep_rows = np.concatenate(
            [shard[:QCOL, 4 * P :], shard[3 * QCOL :, 4 * P :]]
        )
        d8bt = deep_rows.T.reshape(4, P, 2, QCOL)
        d8bc = np.ascontiguousarray(d8bt.transpose(2, 1, 0, 3).astype(E4))
        # d8c: q2 rows (2048:3072), k-blocks 4,5 -- extra deep groups o>=6
        d8ct = shard[2 * QCOL : 3 * QCOL, 4 * P : 6 * P].T.reshape(2, P, QCOL)
        d8cc = np.ascontiguousarray(d8ct.transpose(1, 0, 2).astype(E4))
        in_maps.append(
            {"dT": dTc, "d8a": d8ac, "d8b": d8bc, "d8c": d8cc, "wT": wT,
             "w8lo": w8lo, "w8hi": w8hi, "biasb": bias2}
        )
    return in_maps


def _run(data, W, b, trace=False, **trace_kw):
    nc = _get_nc()
    in_maps = _prep_inputs(data, W, b)
    res = run_bass_kernel_spmd(
        nc, in_maps, list(range(N_CORES)), trace=trace, **trace_kw
    )
    out = np.concatenate(
        [
            np.asarray(res.results[c]["outT"]).T.astype(np.float32)
            for c in range(N_CORES)
        ],
        axis=0,
    )
    return out, res


def kernel(**inputs) -> np.ndarray:
    out, _ = _run(inputs["data"], inputs["W"], inputs["b"])
    return out
